# revision 19
# baseline (speedup 1.0000x reference)
"""Trainium2 Bass kernel for nn_LocalFeatureCluster (vq_codebook).

Data-parallel over batch: each of the 8 NeuronCores runs one sample's full
kmeans + sim-weighted refine + MLP pipeline. Host prepares constants
(sim matrix, init-centroid gather, tiny position-encoder) and shards.
"""
import math
import os
import sys
import types

import numpy as np

# ---------------------------------------------------------------- constants
B, C, HH, WW = 8, 512, 48, 48
N = HH * WW            # 2304
K = 691                # max(2, int(N * 0.3))
C2 = C // 2            # 256
ITERS = 10
NT = N // 128          # 18 n-tiles
CT = C // 128          # 4 c-tiles
KT = 6                 # k-tiles: 5*128 + 51
KROWS = [128, 128, 128, 128, 128, 51]
F32 = np.float32

# ------------------------------------------------------- axon NTFF trace shim
def _install_trace_shim():
    if "antenv.axon_hooks" in sys.modules:
        return
    try:
        mod = types.ModuleType("antenv.axon_hooks")
        mod._hook = None

        def _set(h):
            mod._hook = h

        def _get():
            return mod._hook

        mod.set_axon_ntff_profile_hook = _set
        mod.get_axon_ntff_profile_hook = _get
        sys.modules["antenv.axon_hooks"] = mod
        import antenv

        antenv.axon_hooks = mod
        from trn_agent_boot.trn_boot import _ntff_profile_via_ctypes

        _set(_ntff_profile_via_ctypes("/opt/axon/libaxon_pjrt.so"))
    except Exception:
        pass


_install_trace_shim()

# ------------------------------------------------------------- host compute
def _host_pos_emb(p):
    """EnhancedPositionEncoder forward in float64 -> [N, C] float32."""
    from scipy.special import erf

    ls = np.linspace(-1.0, 1.0, HH).astype(np.float32).astype(np.float64)
    gx = np.broadcast_to(ls[None, :], (HH, WW)).astype(np.float64)  # varies with j
    gy = np.broadcast_to(ls[:, None], (HH, WW)).astype(np.float64)  # varies with i

    def gelu(x):
        return x * 0.5 * (1.0 + erf(x / np.sqrt(2.0)))

    def conv_h(x, w, b):
        # x [ci, H, W], w [co, ci, 3, 1], pad (1,0) along H
        xp = np.pad(x, ((0, 0), (1, 1), (0, 0)))
        col = np.stack([xp[:, k:k + HH, :] for k in range(3)], axis=1)
        y = np.einsum("cihw,oci->ohw", col, w[:, :, :, 0])
        return y + b[:, None, None]

    def conv_w(x, w, b):
        xp = np.pad(x, ((0, 0), (0, 0), (1, 1)))
        col = np.stack([xp[:, :, k:k + WW] for k in range(3)], axis=1)
        y = np.einsum("cihw,oci->ohw", col, w[:, :, 0, :])
        return y + b[:, None, None]

    g = lambda name: np.asarray(p[name], np.float64)
    t = conv_h(gx[None], g("te_w1"), g("te_b1"))
    t = conv_h(gelu(t), g("te_w2"), g("te_b2"))            # [C, H, W]
    f = conv_w(gy[None], g("fe_w1"), g("fe_b1"))
    f = conv_w(gelu(f), g("fe_w2"), g("fe_b2"))            # [C, H, W]
    comb = np.concatenate([t, f], axis=0)                  # [2C, H, W]
    g1 = np.einsum("chw,oc->ohw", comb, g("g_w1")[:, :, 0, 0]) + g("g_b1")[:, None, None]
    g1 = np.maximum(g1, 0.0)
    g2 = np.einsum("chw,oc->ohw", g1, g("g_w2")[:, :, 0, 0]) + g("g_b2")[:, None, None]
    gate = 1.0 / (1.0 + np.exp(-g2))
    pos = gate * t + (1.0 - gate) * f                      # [C, H, W]
    return np.ascontiguousarray(pos.reshape(C, N).T).astype(F32)  # [N, C]


def _host_sim():
    ii, jj = np.meshgrid(np.arange(HH, dtype=np.float64), np.arange(WW, dtype=np.float64), indexing="ij")
    pos2 = np.stack([ii.ravel(), jj.ravel()], axis=1)      # [N, 2]
    d2 = ((pos2[:, None, :] - pos2[None, :, :]) ** 2).sum(-1)
    return np.exp(-np.sqrt(d2)).astype(F32)                # [N, N]


def _host_idx():
    return np.linspace(0, N - 1, K).astype(F32).astype(np.int32)


# ------------------------------------------------------------ device program
_PROG = {}


def _build_program():
    import concourse.bacc as bacc
    import concourse.mybir as mybir
    import concourse.tile as tile
    from concourse.alu_op_type import AluOpType as op

    dt = mybir.dt
    AX = mybir.AxisListType.X
    ACT = mybir.ActivationFunctionType

    nc = bacc.Bacc("TRN2", target_bir_lowering=False, debug=False)

    dti = lambda name, shape: nc.dram_tensor(name, shape, dt.float32, kind="ExternalInput")
    xt_d = dti("xt", [C, N])
    x_d = dti("x", [N, C])
    post_d = dti("post", [C, N])
    pos_d = dti("pos", [N, C])
    sim_d = dti("sim", [N, N])
    c0t_d = dti("c0t", [C, K])
    iotab_d = dti("iotab", [128, K])
    eye_d = dti("eye", [128, 128])
    spw1_d = dti("spw1", [128, 4, C2])
    spb1_d = dti("spb1", [128, 2])
    spw2_d = dti("spw2", [128, 2])
    spb2_d = dti("spb2", [128, 1])
    fuw1_d = dti("fuw1", [128, 8, C2])
    fub1_d = dti("fub1", [128, 2])
    fuw2_d = dti("fuw2", [128, 2])
    fub2_d = dti("fub2", [1, 1])

    outv_d = nc.dram_tensor("outv", [1, C], dt.float32, kind="ExternalOutput")
    cent_d = nc.dram_tensor("centers", [K, C], dt.float32, kind="ExternalOutput")

    with tile.TileContext(nc) as tc:
        pers = tc.alloc_tile_pool(name="pers", bufs=1)
        XT = pers.tile([128, CT, N], dt.float32, tag="XT")
        XE = pers.tile([128, NT, C], dt.float32, tag="XE")
        OH = pers.tile([128, NT, K], dt.float32, tag="OH")
        CENT = pers.tile([128, KT, C], dt.float32, tag="CENT")
        CENTT = pers.tile([128, CT, K], dt.float32, tag="CENTT")
        IOTB = pers.tile([128, K], dt.float32, tag="IOTB")
        EYE = pers.tile([128, 128], dt.float32, tag="EYE")
        LAB = pers.tile([128, NT], dt.float32, tag="LAB")
        SWT = pers.tile([128, NT], dt.float32, tag="SWT")
        WWT = pers.tile([128, NT], dt.float32, tag="WWT")
        CNTSB = pers.tile([1, K], dt.float32, tag="CNTSB")
        CWS = pers.tile([128, KT, 3], dt.float32, tag="CWS")
        ONES1 = pers.tile([128, 1], dt.float32, tag="ONES1")
        ONESR = pers.tile([1, 128], dt.float32, tag="ONESR")
        PSR = pers.tile([1, C], dt.float32, tag="PSR")      # possum row

        nc.vector.memset(ONES1[:], 1.0)
        nc.vector.memset(ONESR[:], 1.0)

        # ---- loads
        nc.sync.dma_start(XT[:], xt_d.ap().rearrange("(a p) f -> p a f", p=128))
        nc.sync.dma_start(XE[:], x_d.ap().rearrange("(t p) f -> p t f", p=128))
        nc.sync.dma_start(CENTT[:], c0t_d.ap().rearrange("(a p) f -> p a f", p=128))
        nc.sync.dma_start(IOTB[:], iotab_d.ap())
        nc.sync.dma_start(EYE[:], eye_d.ap())

        # ---- prologue: enhanced = features + pos_emb (both layouts), possum row
        with tc.tile_pool(name="stream", bufs=2) as st, \
             tc.tile_pool(name="psr", bufs=1, space="PSUM") as psrp:
            for ct in range(CT):
                pt = st.tile([128, N], dt.float32, tag="post")
                nc.sync.dma_start(pt[:], post_d.ap().rearrange("(a p) f -> p a f", p=128)[:, ct, :])
                nc.vector.tensor_tensor(XT[:, ct, :], XT[:, ct, :], pt[:], op.add)
            psp = psrp.tile([1, C], dt.float32, tag="psp")
            for t in range(NT):
                pp = st.tile([128, C], dt.float32, tag="pos")
                nc.sync.dma_start(pp[:], pos_d.ap().rearrange("(t p) f -> p t f", p=128)[:, t, :])
                nc.tensor.matmul(psp[0:1, :], ONES1[:], pp[:], start=(t == 0), stop=(t == NT - 1))
                nc.vector.tensor_tensor(XE[:, t, :], XE[:, t, :], pp[:], op.add)
            nc.scalar.copy(PSR[:], psp[0:1, :])

        # ---- spatial MLP: relu1T = relu(spw1.T @ xeT), sw = sigmoid(...)
        with tc.tile_pool(name="r1", bufs=1) as r1p:
            R1T = r1p.tile([128, 2, N], dt.float32, tag="R1T")
            SPW1 = r1p.tile([128, 4, C2], dt.float32, tag="SPW1")
            SPB1 = r1p.tile([128, 2], dt.float32, tag="SPB1")
            SPW2 = r1p.tile([128, 2], dt.float32, tag="SPW2")
            SPB2 = r1p.tile([128, 1], dt.float32, tag="SPB2")
            nc.sync.dma_start(SPW1[:], spw1_d.ap())
            nc.sync.dma_start(SPB1[:], spb1_d.ap())
            nc.sync.dma_start(SPW2[:], spw2_d.ap())
            nc.sync.dma_start(SPB2[:], spb2_d.ap())
            with tc.tile_pool(name="sp1", bufs=2, space="PSUM") as sp1:
                chunks = [(i * 512, min(512, N - i * 512)) for i in range((N + 511) // 512)]
                for m in range(2):
                    for off, w in chunks:
                        pc = sp1.tile([128, 512], dt.float32, tag="sp")
                        for kt in range(CT):
                            nc.tensor.matmul(pc[:, 0:w], SPW1[:, kt, m * 128:(m + 1) * 128],
                                             XT[:, kt, off:off + w],
                                             start=(kt == 0), stop=(kt == CT - 1))
                        nc.scalar.activation(R1T[:, m, off:off + w], pc[:, 0:w], ACT.Relu,
                                             bias=SPB1[:, m:m + 1])
            with tc.tile_pool(name="sp2", bufs=1, space="PSUM") as sp2:
                swp = sp2.tile([128, NT], dt.float32, tag="swp")
                for t in range(NT):
                    for q in range(2):
                        nc.tensor.matmul(swp[:, t:t + 1], R1T[:, q, t * 128:(t + 1) * 128],
                                         SPW2[:, q:q + 1], start=(q == 0), stop=(q == 1))
                nc.scalar.activation(SWT[:], swp[:], ACT.Sigmoid, bias=SPB2[:, 0:1])

        # ---- kmeans: 10 update iterations + final assignment pass
        with tc.tile_pool(name="small", bufs=4) as small, \
             tc.tile_pool(name="medium", bufs=2) as medium:
            for it in range(ITERS + 1):
                itb = tc.alloc_tile_pool(name=f"itb{it}", bufs=1)
                NEGCC2 = itb.tile([1, K], dt.float32, tag="NEGCC2")
                # negcc2 = -0.5 * colsum(centT^2) as [1, K]
                with tc.tile_pool(name=f"cc{it}", bufs=1, space="PSUM") as ccp_p, \
                     tc.tile_pool(name=f"sq{it}", bufs=2) as sqp:
                    ccp = ccp_p.tile([1, K], dt.float32, tag="ccp")
                    for ct in range(CT):
                        sq = sqp.tile([128, K], dt.float32, tag="sq")
                        nc.scalar.activation(sq[:], CENTT[:, ct, :], ACT.Square)
                        nc.tensor.matmul(ccp[0:1, 0:512], ONES1[:], sq[:, 0:512],
                                         start=(ct == 0), stop=(ct == CT - 1))
                        nc.tensor.matmul(ccp[0:1, 512:K], ONES1[:], sq[:, 512:K],
                                         start=(ct == 0), stop=(ct == CT - 1))
                    nc.scalar.activation(NEGCC2[:], ccp[0:1, :], ACT.Copy, scale=-0.5)

                # distances + argmax(S - cc/2) -> labels, one-hot, counts row
                with tc.tile_pool(name=f"d{it}", bufs=2, space="PSUM") as dps, \
                     tc.tile_pool(name=f"cn{it}", bufs=1, space="PSUM") as cnp:
                    pcnt = cnp.tile([1, K], dt.float32, tag="pcnt")
                    for t in range(NT):
                        S = dps.tile([128, K], dt.float32, tag="S")
                        for ct in range(CT):
                            nc.tensor.matmul(S[:, 0:512], XT[:, ct, t * 128:(t + 1) * 128],
                                             CENTT[:, ct, 0:512], start=(ct == 0), stop=False)
                            nc.tensor.matmul(S[:, 512:K], XT[:, ct, t * 128:(t + 1) * 128],
                                             CENTT[:, ct, 512:K], start=(ct == 0), stop=False)
                        nc.tensor.matmul(S[:, 0:512], ONESR[:], NEGCC2[0:1, 0:512],
                                         start=False, stop=True)
                        nc.tensor.matmul(S[:, 512:K], ONESR[:], NEGCC2[0:1, 512:K],
                                         start=False, stop=True)
                        mx = small.tile([128, 8], dt.float32, tag="mx")
                        mi = small.tile([128, 8], dt.uint32, tag="mi")
                        nc.vector.max(mx[:], S[:])
                        nc.vector.max_index(mi[:], mx[:], S[:])
                        nc.vector.tensor_copy(LAB[:, t:t + 1], mi[:, 0:1])
                        nc.vector.tensor_scalar(OH[:, t, :], IOTB[:], LAB[:, t:t + 1], None, op.is_equal)
                        nc.tensor.matmul(pcnt[0:1, 0:512], ONES1[:], OH[:, t, 0:512],
                                         start=(t == 0), stop=(t == NT - 1))
                        nc.tensor.matmul(pcnt[0:1, 512:K], ONES1[:], OH[:, t, 512:K],
                                         start=(t == 0), stop=(t == NT - 1))
                    nc.scalar.copy(CNTSB[:], pcnt[0:1, :])

                if it == ITERS:
                    itb.release()
                    break

                # broadcast counts, masks
                CNTB = itb.tile([128, K], dt.float32, tag="CNTB")
                CMB = itb.tile([128, K], dt.float32, tag="CMB")
                MGB = itb.tile([128, K], dt.uint8, tag="MGB")
                with tc.tile_pool(name=f"bc{it}", bufs=1, space="PSUM") as bcp:
                    pb = bcp.tile([128, 1024], dt.float32, tag="pb")
                    nc.tensor.matmul(pb[:, 0:512], ONESR[:], CNTSB[0:1, 0:512], start=True, stop=True)
                    nc.tensor.matmul(pb[:, 512:K], ONESR[:], CNTSB[0:1, 512:K], start=True, stop=True)
                    nc.scalar.copy(CNTB[:], pb[:, 0:K])
                nc.vector.tensor_scalar(CMB[:], CNTB[:], 1.0, None, op.max)
                nc.vector.reciprocal(CMB[:], CMB[:])
                nc.vector.tensor_scalar(MGB[:], CNTB[:], 0.0, None, op.is_gt)

                # centroid sums in transposed layout; update CENTT in place
                with tc.tile_pool(name=f"s{it}", bufs=1, space="PSUM") as spsp:
                    PST = spsp.tile([128, CT, 1024], dt.float32, tag="PST")
                    for t in range(NT):
                        for ct in range(CT):
                            nc.tensor.matmul(PST[:, ct, 0:512], XE[:, t, ct * 128:(ct + 1) * 128],
                                             OH[:, t, 0:512], start=(t == 0), stop=(t == NT - 1))
                            nc.tensor.matmul(PST[:, ct, 512:K], XE[:, t, ct * 128:(ct + 1) * 128],
                                             OH[:, t, 512:K], start=(t == 0), stop=(t == NT - 1))
                    for ct in range(CT):
                        q = medium.tile([128, K], dt.float32, tag="q")
                        nc.vector.tensor_tensor(q[:], PST[:, ct, 0:K], CMB[:], op.mult)
                        nc.vector.copy_predicated(CENTT[:, ct, :], MGB[:], q[:])
                itb.release()

            # ---- build CENT [kk, C] from final CENTT
            with tc.tile_pool(name="tr0", bufs=2, space="PSUM") as trp:
                for r in range(KT):
                    rows = KROWS[r]
                    for ct in range(CT):
                        tp = trp.tile([128, 128], dt.float32, tag="tp")
                        nc.tensor.transpose(tp[0:rows, 0:128], CENTT[:, ct, r * 128:r * 128 + rows],
                                            EYE[:, :])
                        nc.scalar.copy(CENT[0:rows, r, ct * 128:(ct + 1) * 128], tp[0:rows, 0:128])

            # ---- refine
            with tc.tile_pool(name="lrow", bufs=1) as lrp:
                LROW = lrp.tile([1, N], dt.float32, tag="LROW")
                LROWB = lrp.tile([128, N], dt.float32, tag="LROWB")
                for t in range(NT):
                    nc.sync.dma_start(LROW[0:1, t * 128:(t + 1) * 128], LAB[:, t:t + 1])
                with tc.tile_pool(name="pbl", bufs=2, space="PSUM") as pbp:
                    for off, w in [(i * 512, min(512, N - i * 512)) for i in range((N + 511) // 512)]:
                        pb2 = pbp.tile([128, 512], dt.float32, tag="pb2")
                        nc.tensor.matmul(pb2[:, 0:w], ONESR[:], LROW[0:1, off:off + w], start=True, stop=True)
                        nc.scalar.copy(LROWB[:, off:off + w], pb2[:, 0:w])

                # per-point weights w_n = sum_{m in cluster(n)} sim[n, m]
                with tc.tile_pool(name="simst", bufs=2) as sst:
                    for t in range(NT):
                        smt = sst.tile([128, N], dt.float32, tag="simt")
                        nc.sync.dma_start(smt[:], sim_d.ap()[t * 128:(t + 1) * 128, :])
                        nc.vector.scalar_tensor_tensor(smt[:], LROWB[:], LAB[:, t:t + 1], smt[:],
                                                       op.is_equal, op.mult,
                                                       accum_out=WWT[:, t:t + 1])

            with tc.tile_pool(name="ref", bufs=1) as refp:
                # V2 = [w | sw] per point
                V2 = refp.tile([128, NT, 2], dt.float32, tag="V2")
                AUXSB = refp.tile([2, K], dt.float32, tag="AUXSB")
                nc.vector.tensor_copy(V2[:, :, 0], WWT[:])
                nc.vector.tensor_copy(V2[:, :, 1], SWT[:])

                # pass a: aux rows [wsum; ssw] = [w | sw].T @ oh
                with tc.tile_pool(name="rsa", bufs=1, space="PSUM") as rpsa:
                    AUX = rpsa.tile([2, K], dt.float32, tag="AUX")
                    for t in range(NT):
                        st_, sp_ = (t == 0), (t == NT - 1)
                        nc.tensor.matmul(AUX[0:2, 0:512], V2[:, t, :], OH[:, t, 0:512],
                                         start=st_, stop=sp_)
                        nc.tensor.matmul(AUX[0:2, 512:K], V2[:, t, :], OH[:, t, 512:K],
                                         start=st_, stop=sp_)
                    nc.scalar.copy(AUXSB[:], AUX[0:2, :])

                # transpose wsum/ssw/cnt rows into per-kk columns
                with tc.tile_pool(name="tr1", bufs=2, space="PSUM") as tr1:
                    for r in range(KT):
                        rows = KROWS[r]
                        sl = slice(r * 128, r * 128 + rows)
                        tpa = tr1.tile([128, 2], dt.float32, tag="tpa")
                        nc.tensor.transpose(tpa[0:rows, 0:2], AUXSB[0:2, sl], EYE[0:2, 0:2])
                        nc.scalar.copy(CWS[0:rows, r, 0:2], tpa[0:rows, 0:2])
                        tpc = tr1.tile([128, 1], dt.float32, tag="tpc")
                        nc.tensor.transpose(tpc[0:rows, 0:1], CNTSB[0:1, sl], EYE[0:1, 0:1])
                        nc.scalar.copy(CWS[0:rows, r, 2:3], tpc[0:rows, 0:1])

                # pass b: weighted sums (kk layout) + blend
                with tc.tile_pool(name="rsb", bufs=1, space="PSUM") as rpsb:
                    RSUM = rpsb.tile([128, KT, C], dt.float32, tag="RSUM")
                    with tc.tile_pool(name="roh", bufs=3) as rohp:
                        for t in range(NT):
                            ohw = rohp.tile([128, K], dt.float32, tag="rohw")
                            nc.vector.tensor_scalar(ohw[:], IOTB[:], LAB[:, t:t + 1], WWT[:, t:t + 1],
                                                    op.is_equal, op.mult)
                            st_, sp_ = (t == 0), (t == NT - 1)
                            for r in range(KT):
                                rows = KROWS[r]
                                nc.tensor.matmul(RSUM[0:rows, r, :], ohw[:, r * 128:r * 128 + rows],
                                                 XE[:, t, :], start=st_, stop=sp_)
                    # centers = where(cnt>0, rsum/(wsum + max(cnt,1)*1e-6), cent)
                    for r in range(KT):
                        rows = KROWS[r]
                        cm = small.tile([128, 1], dt.float32, tag="cm")
                        wsp = small.tile([128, 1], dt.float32, tag="wsp")
                        mg = small.tile([128, 1], dt.float32, tag="mg")
                        dd = medium.tile([128, C], dt.float32, tag="dd")
                        nc.vector.tensor_scalar(cm[0:rows], CWS[0:rows, r, 2:3], 1.0, None, op.max)
                        nc.vector.scalar_tensor_tensor(wsp[0:rows], cm[0:rows], 1e-6,
                                                       CWS[0:rows, r, 0:1], op.mult, op.add)
                        nc.vector.reciprocal(wsp[0:rows], wsp[0:rows])
                        nc.vector.tensor_scalar(mg[0:rows], CWS[0:rows, r, 2:3], 0.0, None, op.is_gt)
                        nc.vector.scalar_tensor_tensor(dd[0:rows, :], RSUM[0:rows, r, :], wsp[0:rows, 0:1],
                                                       CENT[0:rows, r, :], op.mult, op.subtract)
                        nc.vector.scalar_tensor_tensor(CENT[0:rows, r, :], dd[0:rows, :], mg[0:rows, 0:1],
                                                       CENT[0:rows, r, :], op.mult, op.add)
                        nc.sync.dma_start(cent_d.ap()[r * 128:r * 128 + rows, :], CENT[0:rows, r, :])

            # ---- epilogue: means + fusion gate (row layout)
            with tc.tile_pool(name="epb", bufs=1) as epb, \
                 tc.tile_pool(name="ep", bufs=1, space="PSUM") as eps:
                FUW1 = epb.tile([128, 8, C2], dt.float32, tag="FUW1")
                FUB1 = epb.tile([128, 2], dt.float32, tag="FUB1")
                FUW2 = epb.tile([128, 2], dt.float32, tag="FUW2")
                FUB2 = epb.tile([1, 1], dt.float32, tag="FUB2")
                nc.sync.dma_start(FUW1[:], fuw1_d.ap())
                nc.sync.dma_start(FUB1[:], fub1_d.ap())
                nc.sync.dma_start(FUW2[:], fuw2_d.ap())
                nc.sync.dma_start(FUB2[:], fub2_d.ap())
                CER = epb.tile([1, C], dt.float32, tag="CER")
                GROW = epb.tile([1, C], dt.float32, tag="GROW")
                LROW_ = epb.tile([1, C], dt.float32, tag="LROW_")
                T2R = epb.tile([1, C], dt.float32, tag="T2R")
                T3R = epb.tile([1, C], dt.float32, tag="T3R")
                OROW = epb.tile([1, C], dt.float32, tag="OROW")
                GBUF = epb.tile([128, 8], dt.float32, tag="GBUF")
                H2 = epb.tile([128, 2], dt.float32, tag="H2")
                ASB = epb.tile([1, 1], dt.float32, tag="ASB")
                dgl = epb.tile([1, C], dt.float32, tag="dgl")
                cer_p = eps.tile([1, C], dt.float32, tag="cer_p")
                t2_p = eps.tile([1, C], dt.float32, tag="t2_p")
                t3_p = eps.tile([1, C], dt.float32, tag="t3_p")
                for t in range(NT):
                    nc.tensor.matmul(cer_p[0:1, :], ONES1[:], XE[:, t, :],
                                     start=(t == 0), stop=(t == NT - 1))
                    nc.tensor.matmul(t2_p[0:1, :], SWT[:, t:t + 1], XE[:, t, :],
                                     start=(t == 0), stop=(t == NT - 1))
                for r in range(KT):
                    rows = KROWS[r]
                    nc.tensor.matmul(t3_p[0:1, :], CWS[0:rows, r, 1:2], CENT[0:rows, r, :],
                                     start=(r == 0), stop=(r == KT - 1))
                nc.scalar.copy(CER[:], cer_p[0:1, :])
                nc.scalar.copy(T2R[:], t2_p[0:1, :])
                nc.scalar.copy(T3R[:], t3_p[0:1, :])
                # global = (cer - psr)/N ; local = (cer + t3 - t2)/N
                nc.vector.tensor_tensor(GROW[:], CER[:], PSR[:], op.subtract)
                nc.vector.tensor_scalar(GROW[:], GROW[:], 1.0 / N, None, op.mult)
                nc.vector.tensor_tensor(LROW_[:], CER[:], T3R[:], op.add)
                nc.vector.tensor_tensor(LROW_[:], LROW_[:], T2R[:], op.subtract)
                nc.vector.tensor_scalar(LROW_[:], LROW_[:], 1.0 / N, None, op.mult)

                # transpose [g | l] rows into GBUF columns (K-layout for fusion)
                with tc.tile_pool(name="tr2", bufs=2, space="PSUM") as tr2:
                    for j in range(8):
                        src = GROW if j < 4 else LROW_
                        o = (j % 4) * 128
                        tpg = tr2.tile([128, 1], dt.float32, tag="tpg")
                        nc.tensor.transpose(tpg[0:128, 0:1], src[0:1, o:o + 128], EYE[0:1, 0:1])
                        nc.scalar.copy(GBUF[:, j:j + 1], tpg[:, 0:1])

                pfu = eps.tile([128, 2, 512], dt.float32, tag="pfu")
                pa = eps.tile([1, 1], dt.float32, tag="pa")
                for m in range(2):
                    for kt in range(8):
                        nc.tensor.matmul(pfu[:, m, 0:1], FUW1[:, kt, m * 128:(m + 1) * 128],
                                         GBUF[:, kt:kt + 1], start=(kt == 0), stop=(kt == 7))
                    nc.scalar.activation(H2[:, m:m + 1], pfu[:, m, 0:1], ACT.Relu, bias=FUB1[:, m:m + 1])
                for m in range(2):
                    nc.tensor.matmul(pa[0:1, 0:1], H2[:, m:m + 1], FUW2[:, m:m + 1],
                                     start=(m == 0), stop=(m == 1))
                nc.scalar.activation(ASB[:], pa[0:1, 0:1], ACT.Sigmoid, bias=FUB2[0:1, 0:1])
                # out = l + a*(g - l)
                nc.vector.tensor_tensor(dgl[:], GROW[:], LROW_[:], op.subtract)
                nc.vector.scalar_tensor_tensor(OROW[:], dgl[:], ASB[0:1, 0:1], LROW_[:], op.mult, op.add)
                nc.sync.dma_start(outv_d.ap(), OROW[:])

        pers.release()

    nc.compile()
    return nc


def _get_program():
    if "nc" not in _PROG:
        _PROG["nc"] = _build_program()
    return _PROG["nc"]


# ------------------------------------------------------------------- kernel
def _prep_in_maps(features, params):
    feats = np.asarray(features, F32)
    p = {k: np.asarray(v, F32) for k, v in params.items()}

    pos = _host_pos_emb(p)                       # [N, C] f32
    post = np.ascontiguousarray(pos.T)           # [C, N]
    sim = _host_sim()                            # [N, N]
    idx = _host_idx()                            # [K]
    iotab = np.broadcast_to(np.arange(K, dtype=F32), (128, K)).copy()
    eye = np.eye(128, dtype=F32)

    spw1 = np.ascontiguousarray(p["sp_w1"].reshape(4, 128, C2).transpose(1, 0, 2))
    spb1 = np.ascontiguousarray(p["sp_b1"].reshape(2, 128).T)
    spw2 = np.ascontiguousarray(p["sp_w2"].reshape(2, 128).T)
    spb2 = np.full((128, 1), p["sp_b2"][0], F32)
    fuw1 = np.ascontiguousarray(p["fu_w1"].reshape(8, 128, C2).transpose(1, 0, 2))
    fub1 = np.ascontiguousarray(p["fu_b1"].reshape(2, 128).T)
    fuw2 = np.ascontiguousarray(p["fu_w2"].reshape(2, 128).T)
    fub2 = np.full((1, 1), p["fu_b2"][0], F32)

    shared = dict(post=post, pos=pos, sim=sim, iotab=iotab, eye=eye,
                  spw1=spw1, spb1=spb1, spw2=spw2, spb2=spb2,
                  fuw1=fuw1, fub1=fub1, fuw2=fuw2, fub2=fub2)

    in_maps = []
    for b in range(B):
        xt = np.ascontiguousarray(feats[b].reshape(C, N))
        x = np.ascontiguousarray(xt.T)
        xe = x + pos
        c0t = np.ascontiguousarray(xe[idx].T)
        in_maps.append(dict(xt=xt, x=x, c0t=c0t, **shared))
    return in_maps


def kernel(features, params):
    from concourse import bass_utils

    nc = _get_program()
    in_maps = _prep_in_maps(features, params)
    res = bass_utils.run_bass_kernel_spmd(nc, in_maps, core_ids=list(range(B)))
    if res.exec_time_ns is not None:
        print(f"HW exec time: {res.exec_time_ns} ns")

    out = np.empty((B, C), F32)
    centers = np.empty((B, K, C), F32)
    for b, r in enumerate(res.results):
        out[b] = r["outv"].reshape(C)
        centers[b] = r["centers"]
    return out, centers


# revision 27
# speedup vs baseline: 1.7907x; 1.7907x over previous
"""Trainium2 Bass kernel for nn_LocalFeatureCluster (vq_codebook).

Data-parallel over batch: each of the 8 NeuronCores runs one sample's full
kmeans + sim-weighted refine + MLP pipeline. Host prepares constants
(sim matrix, init-centroid gather, tiny position-encoder) and shards.
"""
import math
import os
import sys
import types

import numpy as np

# ---------------------------------------------------------------- constants
B, C, HH, WW = 8, 512, 48, 48
N = HH * WW            # 2304
K = 691                # max(2, int(N * 0.3))
C2 = C // 2            # 256
ITERS = 10
NT = N // 128          # 18 n-tiles
CT = C // 128          # 4 c-tiles
KT = 6                 # k-tiles: 5*128 + 51
KROWS = [128, 128, 128, 128, 128, 51]
F32 = np.float32

# ------------------------------------------------------- axon NTFF trace shim
def _install_trace_shim():
    if "antenv.axon_hooks" in sys.modules:
        return
    try:
        mod = types.ModuleType("antenv.axon_hooks")
        mod._hook = None

        def _set(h):
            mod._hook = h

        def _get():
            return mod._hook

        mod.set_axon_ntff_profile_hook = _set
        mod.get_axon_ntff_profile_hook = _get
        sys.modules["antenv.axon_hooks"] = mod
        import antenv

        antenv.axon_hooks = mod
        from trn_agent_boot.trn_boot import _ntff_profile_via_ctypes

        _set(_ntff_profile_via_ctypes("/opt/axon/libaxon_pjrt.so"))
    except Exception:
        pass


_install_trace_shim()

# ------------------------------------------------------------- host compute
def _host_pos_emb(p):
    """EnhancedPositionEncoder forward in float64 -> [N, C] float32."""
    from scipy.special import erf

    ls = np.linspace(-1.0, 1.0, HH).astype(np.float32).astype(np.float64)
    gx = np.broadcast_to(ls[None, :], (HH, WW)).astype(np.float64)  # varies with j
    gy = np.broadcast_to(ls[:, None], (HH, WW)).astype(np.float64)  # varies with i

    def gelu(x):
        return x * 0.5 * (1.0 + erf(x / np.sqrt(2.0)))

    def conv_h(x, w, b):
        # x [ci, H, W], w [co, ci, 3, 1], pad (1,0) along H
        xp = np.pad(x, ((0, 0), (1, 1), (0, 0)))
        col = np.stack([xp[:, k:k + HH, :] for k in range(3)], axis=1)
        y = np.einsum("cihw,oci->ohw", col, w[:, :, :, 0])
        return y + b[:, None, None]

    def conv_w(x, w, b):
        xp = np.pad(x, ((0, 0), (0, 0), (1, 1)))
        col = np.stack([xp[:, :, k:k + WW] for k in range(3)], axis=1)
        y = np.einsum("cihw,oci->ohw", col, w[:, :, 0, :])
        return y + b[:, None, None]

    g = lambda name: np.asarray(p[name], np.float64)
    t = conv_h(gx[None], g("te_w1"), g("te_b1"))
    t = conv_h(gelu(t), g("te_w2"), g("te_b2"))            # [C, H, W]
    f = conv_w(gy[None], g("fe_w1"), g("fe_b1"))
    f = conv_w(gelu(f), g("fe_w2"), g("fe_b2"))            # [C, H, W]
    comb = np.concatenate([t, f], axis=0)                  # [2C, H, W]
    g1 = np.einsum("chw,oc->ohw", comb, g("g_w1")[:, :, 0, 0]) + g("g_b1")[:, None, None]
    g1 = np.maximum(g1, 0.0)
    g2 = np.einsum("chw,oc->ohw", g1, g("g_w2")[:, :, 0, 0]) + g("g_b2")[:, None, None]
    gate = 1.0 / (1.0 + np.exp(-g2))
    pos = gate * t + (1.0 - gate) * f                      # [C, H, W]
    return np.ascontiguousarray(pos.reshape(C, N).T).astype(F32)  # [N, C]


def _host_sim():
    ii, jj = np.meshgrid(np.arange(HH, dtype=np.float64), np.arange(WW, dtype=np.float64), indexing="ij")
    pos2 = np.stack([ii.ravel(), jj.ravel()], axis=1)      # [N, 2]
    d2 = ((pos2[:, None, :] - pos2[None, :, :]) ** 2).sum(-1)
    return np.exp(-np.sqrt(d2)).astype(F32)                # [N, N]


def _host_idx():
    return np.linspace(0, N - 1, K).astype(F32).astype(np.int32)


# ------------------------------------------------------------ device program
_PROG = {}


def _build_program():
    import concourse.bacc as bacc
    import concourse.mybir as mybir
    import concourse.tile as tile
    from concourse.alu_op_type import AluOpType as op

    dt = mybir.dt
    AX = mybir.AxisListType.X
    ACT = mybir.ActivationFunctionType

    nc = bacc.Bacc("TRN2", target_bir_lowering=False, debug=False)

    dti = lambda name, shape: nc.dram_tensor(name, shape, dt.float32, kind="ExternalInput")
    xt_d = dti("xt", [C, N])
    x_d = dti("x", [N, C])
    post_d = dti("post", [C, N])
    pos_d = dti("pos", [N, C])
    sim_d = dti("sim", [N, N])
    c0t_d = dti("c0t", [C, K])
    iotab_d = dti("iotab", [128, K])
    eye_d = dti("eye", [128, 128])
    spw1_d = dti("spw1", [128, 4, C2])
    spb1_d = dti("spb1", [128, 2])
    spw2_d = dti("spw2", [128, 2])
    spb2_d = dti("spb2", [128, 1])
    fuw1_d = dti("fuw1", [128, 8, C2])
    fub1_d = dti("fub1", [128, 2])
    fuw2_d = dti("fuw2", [128, 2])
    fub2_d = dti("fub2", [1, 1])

    outv_d = nc.dram_tensor("outv", [1, C], dt.float32, kind="ExternalOutput")
    cent_d = nc.dram_tensor("centers", [K, C], dt.float32, kind="ExternalOutput")

    with tile.TileContext(nc) as tc:
        pers = tc.alloc_tile_pool(name="pers", bufs=1)
        XH1 = pers.tile([128, CT, N], dt.float16, tag="XH1")
        XH2 = pers.tile([128, CT, N], dt.float16, tag="XH2")
        EH1 = pers.tile([128, NT, C], dt.float16, tag="EH1")
        EH2 = pers.tile([128, NT, C], dt.float16, tag="EH2")
        GH1 = pers.tile([128, CT, K], dt.float16, tag="GH1")
        GH2 = pers.tile([128, CT, K], dt.float16, tag="GH2")
        OH = pers.tile([128, NT, K], dt.float16, tag="OH")
        ONES1H = pers.tile([128, 1], dt.float16, tag="ONES1H")
        V2H = pers.tile([128, NT, 4], dt.float16, tag="V2H")
        CENT = pers.tile([128, KT, C], dt.float32, tag="CENT")
        CENTT = pers.tile([128, CT, K], dt.float32, tag="CENTT")
        IOTB = pers.tile([128, K], dt.float32, tag="IOTB")
        EYE = pers.tile([128, 128], dt.float32, tag="EYE")
        LAB = pers.tile([128, NT], dt.float32, tag="LAB")
        SWT = pers.tile([128, NT], dt.float32, tag="SWT")
        WWT = pers.tile([128, NT], dt.float32, tag="WWT")
        WHIF = pers.tile([128, NT], dt.float32, tag="WHIF")
        WLOF = pers.tile([128, NT], dt.float32, tag="WLOF")
        CNTSB = pers.tile([1, K], dt.float32, tag="CNTSB")
        CWS = pers.tile([128, KT, 5], dt.float32, tag="CWS")
        SSWC = pers.tile([128, KT], dt.float32, tag="SSWC")
        ONES1 = pers.tile([128, 1], dt.float32, tag="ONES1")
        ONESR = pers.tile([1, 128], dt.float32, tag="ONESR")
        PSR = pers.tile([1, C], dt.float32, tag="PSR")      # possum row

        nc.vector.memset(ONES1[:], 1.0)
        nc.vector.memset(ONES1H[:], 1.0)
        nc.vector.memset(ONESR[:], 1.0)
        nc.vector.memset(CWS[:], 0.0)

        # ---- loads
        nc.sync.dma_start(CENTT[:], c0t_d.ap().rearrange("(a p) f -> p a f", p=128))
        nc.sync.dma_start(IOTB[:], iotab_d.ap())
        nc.sync.dma_start(EYE[:], eye_d.ap())

        # ---- prologue: enhanced = features + pos_emb, fp16 hi/lo splits, possum
        with tc.tile_pool(name="stream", bufs=2) as st, \
             tc.tile_pool(name="psr", bufs=1, space="PSUM") as psrp:
            for ct in range(CT):
                xtf = st.tile([128, N], dt.float32, tag="xtf")
                nc.sync.dma_start(xtf[:], xt_d.ap().rearrange("(a p) f -> p a f", p=128)[:, ct, :])
                pt = st.tile([128, N], dt.float32, tag="post")
                nc.sync.dma_start(pt[:], post_d.ap().rearrange("(a p) f -> p a f", p=128)[:, ct, :])
                nc.vector.tensor_tensor(xtf[:], xtf[:], pt[:], op.add)
                nc.vector.tensor_copy(XH1[:, ct, :], xtf[:])
                nc.scalar.copy(pt[:], XH1[:, ct, :])
                nc.vector.tensor_tensor(xtf[:], xtf[:], pt[:], op.subtract)
                nc.vector.tensor_copy(XH2[:, ct, :], xtf[:])
            psp = psrp.tile([1, C], dt.float32, tag="psp")
            for t in range(NT):
                xef = st.tile([128, C], dt.float32, tag="xef")
                nc.sync.dma_start(xef[:], x_d.ap().rearrange("(t p) f -> p t f", p=128)[:, t, :])
                pp = st.tile([128, C], dt.float32, tag="pos")
                nc.sync.dma_start(pp[:], pos_d.ap().rearrange("(t p) f -> p t f", p=128)[:, t, :])
                nc.tensor.matmul(psp[0:1, :], ONES1[:], pp[:], start=(t == 0), stop=(t == NT - 1))
                nc.vector.tensor_tensor(xef[:], xef[:], pp[:], op.add)
                nc.vector.tensor_copy(EH1[:, t, :], xef[:])
                nc.scalar.copy(pp[:], EH1[:, t, :])
                nc.vector.tensor_tensor(xef[:], xef[:], pp[:], op.subtract)
                nc.vector.tensor_copy(EH2[:, t, :], xef[:])
            nc.scalar.copy(PSR[:], psp[0:1, :])

        # ---- spatial MLP (fp16): relu1T = relu(spw1.T @ xeT), sw = sigmoid(...)
        with tc.tile_pool(name="r1", bufs=1) as r1p:
            R1T = r1p.tile([128, 2, N], dt.float16, tag="R1T")
            SPW1 = r1p.tile([128, 4, C2], dt.float32, tag="SPW1")
            SPW1H = r1p.tile([128, 4, C2], dt.float16, tag="SPW1H")
            SPB1 = r1p.tile([128, 2], dt.float32, tag="SPB1")
            SPW2 = r1p.tile([128, 2], dt.float32, tag="SPW2")
            SPW2H = r1p.tile([128, 2], dt.float16, tag="SPW2H")
            SPB2 = r1p.tile([128, 1], dt.float32, tag="SPB2")
            nc.sync.dma_start(SPW1[:], spw1_d.ap())
            nc.sync.dma_start(SPB1[:], spb1_d.ap())
            nc.sync.dma_start(SPW2[:], spw2_d.ap())
            nc.sync.dma_start(SPB2[:], spb2_d.ap())
            nc.vector.tensor_copy(SPW1H[:], SPW1[:])
            nc.vector.tensor_copy(SPW2H[:], SPW2[:])
            with tc.tile_pool(name="sp1", bufs=2, space="PSUM") as sp1:
                chunks = [(i * 512, min(512, N - i * 512)) for i in range((N + 511) // 512)]
                for m in range(2):
                    for off, w in chunks:
                        pc = sp1.tile([128, 512], dt.float32, tag="sp")
                        for kt in range(CT):
                            nc.tensor.matmul(pc[:, 0:w], SPW1H[:, kt, m * 128:(m + 1) * 128],
                                             XH1[:, kt, off:off + w],
                                             start=(kt == 0), stop=(kt == CT - 1))
                        nc.scalar.activation(R1T[:, m, off:off + w], pc[:, 0:w], ACT.Relu,
                                             bias=SPB1[:, m:m + 1])
            with tc.tile_pool(name="sp2", bufs=1, space="PSUM") as sp2:
                swp = sp2.tile([128, NT], dt.float32, tag="swp")
                for t in range(NT):
                    for q in range(2):
                        nc.tensor.matmul(swp[:, t:t + 1], R1T[:, q, t * 128:(t + 1) * 128],
                                         SPW2H[:, q:q + 1], start=(q == 0), stop=(q == 1))
                nc.scalar.activation(SWT[:], swp[:], ACT.Sigmoid, bias=SPB2[:, 0:1])
                nc.vector.tensor_copy(V2H[:, :, 2], SWT[:])
                nc.scalar.copy(WHIF[:], V2H[:, :, 2])
                nc.vector.tensor_tensor(WLOF[:], SWT[:], WHIF[:], op.subtract)
                nc.vector.tensor_copy(V2H[:, :, 3], WLOF[:])

        # ---- kmeans: 10 update iterations + final assignment pass
        with tc.tile_pool(name="small", bufs=4) as small, \
             tc.tile_pool(name="medium", bufs=2) as medium:
            for it in range(ITERS + 1):
                itb = tc.alloc_tile_pool(name=f"itb{it}", bufs=1)
                CCB = itb.tile([128, K], dt.float32, tag="CCB")
                CCH = itb.tile([1, K], dt.float32, tag="CCH")
                # fp16 hi/lo split of current centroids
                for ct in range(CT):
                    gt = medium.tile([128, K], dt.float32, tag="gt")
                    nc.vector.tensor_copy(GH1[:, ct, :], CENTT[:, ct, :])
                    nc.scalar.copy(gt[:], GH1[:, ct, :])
                    nc.vector.tensor_tensor(gt[:], CENTT[:, ct, :], gt[:], op.subtract)
                    nc.vector.tensor_copy(GH2[:, ct, :], gt[:])
                # ccb = broadcast of 0.5*colsum(centT^2) [128, K]
                with tc.tile_pool(name=f"cc{it}", bufs=1, space="PSUM") as ccp_p, \
                     tc.tile_pool(name=f"sq{it}", bufs=2) as sqp:
                    ccp = ccp_p.tile([1, K], dt.float32, tag="ccp")
                    for ct in range(CT):
                        sq = sqp.tile([128, K], dt.float32, tag="sq")
                        nc.scalar.activation(sq[:], CENTT[:, ct, :], ACT.Square)
                        nc.tensor.matmul(ccp[0:1, 0:512], ONES1[:], sq[:, 0:512],
                                         start=(ct == 0), stop=(ct == CT - 1))
                        nc.tensor.matmul(ccp[0:1, 512:K], ONES1[:], sq[:, 512:K],
                                         start=(ct == 0), stop=(ct == CT - 1))
                    nc.scalar.activation(CCH[:], ccp[0:1, :], ACT.Copy, scale=0.5)
                with tc.tile_pool(name=f"cb{it}", bufs=1, space="PSUM") as cbp:
                    pcb = cbp.tile([128, 1024], dt.float32, tag="pcb")
                    nc.tensor.matmul(pcb[:, 0:512], ONESR[:], CCH[0:1, 0:512], start=True, stop=True)
                    nc.tensor.matmul(pcb[:, 512:K], ONESR[:], CCH[0:1, 512:K], start=True, stop=True)
                    nc.scalar.copy(CCB[:], pcb[:, 0:K])

                # distances (fp16 3-term) + argmax(S - cc/2) -> labels, one-hot, counts
                with tc.tile_pool(name=f"d{it}", bufs=2, space="PSUM") as dps, \
                     tc.tile_pool(name=f"cn{it}", bufs=1, space="PSUM") as cnp:
                    pcnt = cnp.tile([1, K], dt.float32, tag="pcnt")
                    for t in range(NT):
                        S = dps.tile([128, K], dt.float32, tag="S")
                        tb = slice(t * 128, (t + 1) * 128)
                        pairs = [(XH1, GH1), (XH1, GH2), (XH2, GH1)]
                        for ct in range(CT):
                            for pi, (xh, gh) in enumerate(pairs):
                                st_ = (ct == 0 and pi == 0)
                                sp_ = (ct == CT - 1 and pi == 2)
                                nc.tensor.matmul(S[:, 0:512], xh[:, ct, tb], gh[:, ct, 0:512],
                                                 start=st_, stop=sp_)
                                nc.tensor.matmul(S[:, 512:K], xh[:, ct, tb], gh[:, ct, 512:K],
                                                 start=st_, stop=sp_)
                        dm = medium.tile([128, K], dt.float32, tag="dm")
                        nc.vector.tensor_tensor(dm[:], S[:, 0:K], CCB[:], op.subtract)
                        mx = small.tile([128, 8], dt.float32, tag="mx")
                        mi = small.tile([128, 8], dt.uint32, tag="mi")
                        nc.vector.max(mx[:], dm[:])
                        nc.vector.max_index(mi[:], mx[:], dm[:])
                        nc.vector.tensor_copy(LAB[:, t:t + 1], mi[:, 0:1])
                        nc.vector.tensor_scalar(OH[:, t, :], IOTB[:], LAB[:, t:t + 1], None, op.is_equal)
                        nc.tensor.matmul(pcnt[0:1, 0:512], ONES1H[:], OH[:, t, 0:512],
                                         start=(t == 0), stop=(t == NT - 1))
                        nc.tensor.matmul(pcnt[0:1, 512:K], ONES1H[:], OH[:, t, 512:K],
                                         start=(t == 0), stop=(t == NT - 1))
                    nc.scalar.copy(CNTSB[:], pcnt[0:1, :])

                if it == ITERS:
                    itb.release()
                    break

                # broadcast counts, masks
                CNTB = itb.tile([128, K], dt.float32, tag="CNTB")
                CMB = itb.tile([128, K], dt.float32, tag="CMB")
                MGB = itb.tile([128, K], dt.uint8, tag="MGB")
                with tc.tile_pool(name=f"bc{it}", bufs=1, space="PSUM") as bcp:
                    pb = bcp.tile([128, 1024], dt.float32, tag="pb")
                    nc.tensor.matmul(pb[:, 0:512], ONESR[:], CNTSB[0:1, 0:512], start=True, stop=True)
                    nc.tensor.matmul(pb[:, 512:K], ONESR[:], CNTSB[0:1, 512:K], start=True, stop=True)
                    nc.scalar.copy(CNTB[:], pb[:, 0:K])
                nc.vector.tensor_scalar(CMB[:], CNTB[:], 1.0, None, op.max)
                nc.vector.reciprocal(CMB[:], CMB[:])
                nc.vector.tensor_scalar(MGB[:], CNTB[:], 0.0, None, op.is_gt)

                # centroid sums in transposed layout; update CENTT in place
                with tc.tile_pool(name=f"s{it}", bufs=1, space="PSUM") as spsp:
                    PST = spsp.tile([128, CT, 1024], dt.float32, tag="PST")
                    for t in range(NT):
                        for ct in range(CT):
                            cb = slice(ct * 128, (ct + 1) * 128)
                            for ei, eh in enumerate((EH1, EH2)):
                                st_ = (t == 0 and ei == 0)
                                sp_ = (t == NT - 1 and ei == 1)
                                nc.tensor.matmul(PST[:, ct, 0:512], eh[:, t, cb],
                                                 OH[:, t, 0:512], start=st_, stop=sp_)
                                nc.tensor.matmul(PST[:, ct, 512:K], eh[:, t, cb],
                                                 OH[:, t, 512:K], start=st_, stop=sp_)
                    for ct in range(CT):
                        q = medium.tile([128, K], dt.float32, tag="q")
                        nc.vector.tensor_tensor(q[:], PST[:, ct, 0:K], CMB[:], op.mult)
                        nc.vector.copy_predicated(CENTT[:, ct, :], MGB[:], q[:])
                itb.release()

            # ---- build CENT [kk, C] from final CENTT
            with tc.tile_pool(name="tr0", bufs=2, space="PSUM") as trp:
                for r in range(KT):
                    rows = KROWS[r]
                    for ct in range(CT):
                        tp = trp.tile([128, 128], dt.float32, tag="tp")
                        nc.tensor.transpose(tp[0:rows, 0:128], CENTT[:, ct, r * 128:r * 128 + rows],
                                            EYE[:, :])
                        nc.scalar.copy(CENT[0:rows, r, ct * 128:(ct + 1) * 128], tp[0:rows, 0:128])

            # ---- refine
            with tc.tile_pool(name="lrow", bufs=1) as lrp:
                LROW = lrp.tile([1, N], dt.float32, tag="LROW")
                LROWB = lrp.tile([128, N], dt.float32, tag="LROWB")
                for t in range(NT):
                    nc.sync.dma_start(LROW[0:1, t * 128:(t + 1) * 128], LAB[:, t:t + 1])
                with tc.tile_pool(name="pbl", bufs=2, space="PSUM") as pbp:
                    for off, w in [(i * 512, min(512, N - i * 512)) for i in range((N + 511) // 512)]:
                        pb2 = pbp.tile([128, 512], dt.float32, tag="pb2")
                        nc.tensor.matmul(pb2[:, 0:w], ONESR[:], LROW[0:1, off:off + w], start=True, stop=True)
                        nc.scalar.copy(LROWB[:, off:off + w], pb2[:, 0:w])

                # per-point weights w_n = sum_{m in cluster(n)} sim[n, m]
                with tc.tile_pool(name="simst", bufs=2) as sst:
                    for t in range(NT):
                        smt = sst.tile([128, N], dt.float32, tag="simt")
                        nc.sync.dma_start(smt[:], sim_d.ap()[t * 128:(t + 1) * 128, :])
                        nc.vector.scalar_tensor_tensor(smt[:], LROWB[:], LAB[:, t:t + 1], smt[:],
                                                       op.is_equal, op.mult,
                                                       accum_out=WWT[:, t:t + 1])

            with tc.tile_pool(name="ref", bufs=1) as refp:
                # fp16 hi/lo split of per-point weights w into V2H cols 0,1
                AUXSB = refp.tile([4, K], dt.float32, tag="AUXSB")
                nc.vector.tensor_copy(V2H[:, :, 0], WWT[:])
                nc.scalar.copy(WHIF[:], V2H[:, :, 0])
                nc.vector.tensor_tensor(WLOF[:], WWT[:], WHIF[:], op.subtract)
                nc.vector.tensor_copy(V2H[:, :, 1], WLOF[:])

                # pass a: aux rows [whi; wlo; swhi; swlo].T @ oh
                with tc.tile_pool(name="rsa", bufs=1, space="PSUM") as rpsa:
                    AUX = rpsa.tile([4, K], dt.float32, tag="AUX")
                    for t in range(NT):
                        st_, sp_ = (t == 0), (t == NT - 1)
                        nc.tensor.matmul(AUX[0:4, 0:512], V2H[:, t, :], OH[:, t, 0:512],
                                         start=st_, stop=sp_)
                        nc.tensor.matmul(AUX[0:4, 512:K], V2H[:, t, :], OH[:, t, 512:K],
                                         start=st_, stop=sp_)
                    nc.scalar.copy(AUXSB[:], AUX[0:4, :])

                # transpose aux/cnt rows into per-kk columns
                with tc.tile_pool(name="tr1", bufs=2, space="PSUM") as tr1:
                    for r in range(KT):
                        rows = KROWS[r]
                        sl = slice(r * 128, r * 128 + rows)
                        tpa = tr1.tile([128, 4], dt.float32, tag="tpa")
                        nc.tensor.transpose(tpa[0:rows, 0:4], AUXSB[0:4, sl], EYE[0:4, 0:4])
                        nc.scalar.copy(CWS[0:rows, r, 0:4], tpa[0:rows, 0:4])
                        tpc = tr1.tile([128, 1], dt.float32, tag="tpc")
                        nc.tensor.transpose(tpc[0:rows, 0:1], CNTSB[0:1, sl], EYE[0:1, 0:1])
                        nc.scalar.copy(CWS[0:rows, r, 4:5], tpc[0:rows, 0:1])
                nc.vector.tensor_tensor(SSWC[:], CWS[:, :, 2], CWS[:, :, 3], op.add)

                # pass b: weighted sums (kk layout, fp16 splits) + blend
                with tc.tile_pool(name="rsb", bufs=1, space="PSUM") as rpsb:
                    RSUM = rpsb.tile([128, KT, C], dt.float32, tag="RSUM")
                    with tc.tile_pool(name="roh", bufs=3) as rohp:
                        for t in range(NT):
                            ohwh = rohp.tile([128, K], dt.float16, tag="rohwh")
                            ohwl = rohp.tile([128, K], dt.float16, tag="rohwl")
                            nc.vector.tensor_scalar(ohwh[:], IOTB[:], LAB[:, t:t + 1], WHIF[:, t:t + 1],
                                                    op.is_equal, op.mult)
                            nc.vector.tensor_scalar(ohwl[:], IOTB[:], LAB[:, t:t + 1], WLOF[:, t:t + 1],
                                                    op.is_equal, op.mult)
                            st_, sp_ = (t == 0), (t == NT - 1)
                            terms = [(ohwh, EH1), (ohwh, EH2), (ohwl, EH1)]
                            for r in range(KT):
                                rows = KROWS[r]
                                for pi, (ow, eh) in enumerate(terms):
                                    nc.tensor.matmul(RSUM[0:rows, r, :], ow[:, r * 128:r * 128 + rows],
                                                     eh[:, t, :], start=(st_ and pi == 0),
                                                     stop=(sp_ and pi == 2))
                    # centers = where(cnt>0, rsum/(wsum + max(cnt,1)*1e-6), cent)
                    for r in range(KT):
                        rows = KROWS[r]
                        cm = small.tile([128, 1], dt.float32, tag="cm")
                        ws2 = small.tile([128, 1], dt.float32, tag="ws2")
                        wsp = small.tile([128, 1], dt.float32, tag="wsp")
                        mg = small.tile([128, 1], dt.float32, tag="mg")
                        dd = medium.tile([128, C], dt.float32, tag="dd")
                        nc.vector.tensor_scalar(cm[0:rows], CWS[0:rows, r, 4:5], 1.0, None, op.max)
                        nc.vector.tensor_tensor(ws2[0:rows], CWS[0:rows, r, 0:1], CWS[0:rows, r, 1:2], op.add)
                        nc.vector.scalar_tensor_tensor(wsp[0:rows], cm[0:rows], 1e-6,
                                                       ws2[0:rows, 0:1], op.mult, op.add)
                        nc.vector.reciprocal(wsp[0:rows], wsp[0:rows])
                        nc.vector.tensor_scalar(mg[0:rows], CWS[0:rows, r, 4:5], 0.0, None, op.is_gt)
                        nc.vector.scalar_tensor_tensor(dd[0:rows, :], RSUM[0:rows, r, :], wsp[0:rows, 0:1],
                                                       CENT[0:rows, r, :], op.mult, op.subtract)
                        nc.vector.scalar_tensor_tensor(CENT[0:rows, r, :], dd[0:rows, :], mg[0:rows, 0:1],
                                                       CENT[0:rows, r, :], op.mult, op.add)
                        nc.sync.dma_start(cent_d.ap()[r * 128:r * 128 + rows, :], CENT[0:rows, r, :])

            # ---- epilogue: means + fusion gate (row layout)
            with tc.tile_pool(name="epb", bufs=1) as epb, \
                 tc.tile_pool(name="ep", bufs=1, space="PSUM") as eps:
                FUW1 = epb.tile([128, 8, C2], dt.float32, tag="FUW1")
                FUB1 = epb.tile([128, 2], dt.float32, tag="FUB1")
                FUW2 = epb.tile([128, 2], dt.float32, tag="FUW2")
                FUB2 = epb.tile([1, 1], dt.float32, tag="FUB2")
                nc.sync.dma_start(FUW1[:], fuw1_d.ap())
                nc.sync.dma_start(FUB1[:], fub1_d.ap())
                nc.sync.dma_start(FUW2[:], fuw2_d.ap())
                nc.sync.dma_start(FUB2[:], fub2_d.ap())
                CER = epb.tile([1, C], dt.float32, tag="CER")
                GROW = epb.tile([1, C], dt.float32, tag="GROW")
                LROW_ = epb.tile([1, C], dt.float32, tag="LROW_")
                T2R = epb.tile([1, C], dt.float32, tag="T2R")
                T3R = epb.tile([1, C], dt.float32, tag="T3R")
                OROW = epb.tile([1, C], dt.float32, tag="OROW")
                GBUF = epb.tile([128, 8], dt.float32, tag="GBUF")
                H2 = epb.tile([128, 2], dt.float32, tag="H2")
                ASB = epb.tile([1, 1], dt.float32, tag="ASB")
                dgl = epb.tile([1, C], dt.float32, tag="dgl")
                cer_p = eps.tile([1, C], dt.float32, tag="cer_p")
                t2_p = eps.tile([1, C], dt.float32, tag="t2_p")
                t3_p = eps.tile([1, C], dt.float32, tag="t3_p")
                for t in range(NT):
                    for ei, eh in enumerate((EH1, EH2)):
                        nc.tensor.matmul(cer_p[0:1, :], ONES1H[:], eh[:, t, :],
                                         start=(t == 0 and ei == 0), stop=(t == NT - 1 and ei == 1))
                    t2terms = [(2, EH1), (2, EH2), (3, EH1)]
                    for pi, (col, eh) in enumerate(t2terms):
                        nc.tensor.matmul(t2_p[0:1, :], V2H[:, t, col:col + 1], eh[:, t, :],
                                         start=(t == 0 and pi == 0), stop=(t == NT - 1 and pi == 2))
                for r in range(KT):
                    rows = KROWS[r]
                    nc.tensor.matmul(t3_p[0:1, :], SSWC[0:rows, r:r + 1], CENT[0:rows, r, :],
                                     start=(r == 0), stop=(r == KT - 1))
                nc.scalar.copy(CER[:], cer_p[0:1, :])
                nc.scalar.copy(T2R[:], t2_p[0:1, :])
                nc.scalar.copy(T3R[:], t3_p[0:1, :])
                # global = (cer - psr)/N ; local = (cer + t3 - t2)/N
                nc.vector.tensor_tensor(GROW[:], CER[:], PSR[:], op.subtract)
                nc.vector.tensor_scalar(GROW[:], GROW[:], 1.0 / N, None, op.mult)
                nc.vector.tensor_tensor(LROW_[:], CER[:], T3R[:], op.add)
                nc.vector.tensor_tensor(LROW_[:], LROW_[:], T2R[:], op.subtract)
                nc.vector.tensor_scalar(LROW_[:], LROW_[:], 1.0 / N, None, op.mult)

                # transpose [g | l] rows into GBUF columns (K-layout for fusion)
                with tc.tile_pool(name="tr2", bufs=2, space="PSUM") as tr2:
                    for j in range(8):
                        src = GROW if j < 4 else LROW_
                        o = (j % 4) * 128
                        tpg = tr2.tile([128, 1], dt.float32, tag="tpg")
                        nc.tensor.transpose(tpg[0:128, 0:1], src[0:1, o:o + 128], EYE[0:1, 0:1])
                        nc.scalar.copy(GBUF[:, j:j + 1], tpg[:, 0:1])

                pfu = eps.tile([128, 2, 512], dt.float32, tag="pfu")
                pa = eps.tile([1, 1], dt.float32, tag="pa")
                for m in range(2):
                    for kt in range(8):
                        nc.tensor.matmul(pfu[:, m, 0:1], FUW1[:, kt, m * 128:(m + 1) * 128],
                                         GBUF[:, kt:kt + 1], start=(kt == 0), stop=(kt == 7))
                    nc.scalar.activation(H2[:, m:m + 1], pfu[:, m, 0:1], ACT.Relu, bias=FUB1[:, m:m + 1])
                for m in range(2):
                    nc.tensor.matmul(pa[0:1, 0:1], H2[:, m:m + 1], FUW2[:, m:m + 1],
                                     start=(m == 0), stop=(m == 1))
                nc.scalar.activation(ASB[:], pa[0:1, 0:1], ACT.Sigmoid, bias=FUB2[0:1, 0:1])
                # out = l + a*(g - l)
                nc.vector.tensor_tensor(dgl[:], GROW[:], LROW_[:], op.subtract)
                nc.vector.scalar_tensor_tensor(OROW[:], dgl[:], ASB[0:1, 0:1], LROW_[:], op.mult, op.add)
                nc.sync.dma_start(outv_d.ap(), OROW[:])

        pers.release()

    nc.compile()
    return nc


def _get_program():
    if "nc" not in _PROG:
        _PROG["nc"] = _build_program()
    return _PROG["nc"]


# ------------------------------------------------------------------- kernel
def _prep_in_maps(features, params):
    feats = np.asarray(features, F32)
    p = {k: np.asarray(v, F32) for k, v in params.items()}

    pos = _host_pos_emb(p)                       # [N, C] f32
    post = np.ascontiguousarray(pos.T)           # [C, N]
    sim = _host_sim()                            # [N, N]
    idx = _host_idx()                            # [K]
    iotab = np.broadcast_to(np.arange(K, dtype=F32), (128, K)).copy()
    eye = np.eye(128, dtype=F32)

    spw1 = np.ascontiguousarray(p["sp_w1"].reshape(4, 128, C2).transpose(1, 0, 2))
    spb1 = np.ascontiguousarray(p["sp_b1"].reshape(2, 128).T)
    spw2 = np.ascontiguousarray(p["sp_w2"].reshape(2, 128).T)
    spb2 = np.full((128, 1), p["sp_b2"][0], F32)
    fuw1 = np.ascontiguousarray(p["fu_w1"].reshape(8, 128, C2).transpose(1, 0, 2))
    fub1 = np.ascontiguousarray(p["fu_b1"].reshape(2, 128).T)
    fuw2 = np.ascontiguousarray(p["fu_w2"].reshape(2, 128).T)
    fub2 = np.full((1, 1), p["fu_b2"][0], F32)

    shared = dict(post=post, pos=pos, sim=sim, iotab=iotab, eye=eye,
                  spw1=spw1, spb1=spb1, spw2=spw2, spb2=spb2,
                  fuw1=fuw1, fub1=fub1, fuw2=fuw2, fub2=fub2)

    in_maps = []
    for b in range(B):
        xt = np.ascontiguousarray(feats[b].reshape(C, N))
        x = np.ascontiguousarray(xt.T)
        xe = x + pos
        c0t = np.ascontiguousarray(xe[idx].T)
        in_maps.append(dict(xt=xt, x=x, c0t=c0t, **shared))
    return in_maps


def kernel(features, params):
    from concourse import bass_utils

    nc = _get_program()
    in_maps = _prep_in_maps(features, params)
    res = bass_utils.run_bass_kernel_spmd(nc, in_maps, core_ids=list(range(B)))
    if res.exec_time_ns is not None:
        print(f"HW exec time: {res.exec_time_ns} ns")

    out = np.empty((B, C), F32)
    centers = np.empty((B, K, C), F32)
    for b, r in enumerate(res.results):
        out[b] = r["outv"].reshape(C)
        centers[b] = r["centers"]
    return out, centers


# revision 28
# speedup vs baseline: 1.8745x; 1.0468x over previous
"""Trainium2 Bass kernel for nn_LocalFeatureCluster (vq_codebook).

Data-parallel over batch: each of the 8 NeuronCores runs one sample's full
kmeans + sim-weighted refine + MLP pipeline. Host prepares constants
(sim matrix, init-centroid gather, tiny position-encoder) and shards.
"""
import math
import os
import sys
import types

import numpy as np

# ---------------------------------------------------------------- constants
B, C, HH, WW = 8, 512, 48, 48
N = HH * WW            # 2304
K = 691                # max(2, int(N * 0.3))
C2 = C // 2            # 256
ITERS = 10
NT = N // 128          # 18 n-tiles
CT = C // 128          # 4 c-tiles
KT = 6                 # k-tiles: 5*128 + 51
KROWS = [128, 128, 128, 128, 128, 51]
F32 = np.float32

# ------------------------------------------------------- axon NTFF trace shim
def _install_trace_shim():
    if "antenv.axon_hooks" in sys.modules:
        return
    try:
        mod = types.ModuleType("antenv.axon_hooks")
        mod._hook = None

        def _set(h):
            mod._hook = h

        def _get():
            return mod._hook

        mod.set_axon_ntff_profile_hook = _set
        mod.get_axon_ntff_profile_hook = _get
        sys.modules["antenv.axon_hooks"] = mod
        import antenv

        antenv.axon_hooks = mod
        from trn_agent_boot.trn_boot import _ntff_profile_via_ctypes

        _set(_ntff_profile_via_ctypes("/opt/axon/libaxon_pjrt.so"))
    except Exception:
        pass


_install_trace_shim()

# ------------------------------------------------------------- host compute
def _host_pos_emb(p):
    """EnhancedPositionEncoder forward in float64 -> [N, C] float32."""
    from scipy.special import erf

    ls = np.linspace(-1.0, 1.0, HH).astype(np.float32).astype(np.float64)
    gx = np.broadcast_to(ls[None, :], (HH, WW)).astype(np.float64)  # varies with j
    gy = np.broadcast_to(ls[:, None], (HH, WW)).astype(np.float64)  # varies with i

    def gelu(x):
        return x * 0.5 * (1.0 + erf(x / np.sqrt(2.0)))

    def conv_h(x, w, b):
        # x [ci, H, W], w [co, ci, 3, 1], pad (1,0) along H
        xp = np.pad(x, ((0, 0), (1, 1), (0, 0)))
        col = np.stack([xp[:, k:k + HH, :] for k in range(3)], axis=1)
        y = np.einsum("cihw,oci->ohw", col, w[:, :, :, 0])
        return y + b[:, None, None]

    def conv_w(x, w, b):
        xp = np.pad(x, ((0, 0), (0, 0), (1, 1)))
        col = np.stack([xp[:, :, k:k + WW] for k in range(3)], axis=1)
        y = np.einsum("cihw,oci->ohw", col, w[:, :, 0, :])
        return y + b[:, None, None]

    g = lambda name: np.asarray(p[name], np.float64)
    t = conv_h(gx[None], g("te_w1"), g("te_b1"))
    t = conv_h(gelu(t), g("te_w2"), g("te_b2"))            # [C, H, W]
    f = conv_w(gy[None], g("fe_w1"), g("fe_b1"))
    f = conv_w(gelu(f), g("fe_w2"), g("fe_b2"))            # [C, H, W]
    comb = np.concatenate([t, f], axis=0)                  # [2C, H, W]
    g1 = np.einsum("chw,oc->ohw", comb, g("g_w1")[:, :, 0, 0]) + g("g_b1")[:, None, None]
    g1 = np.maximum(g1, 0.0)
    g2 = np.einsum("chw,oc->ohw", g1, g("g_w2")[:, :, 0, 0]) + g("g_b2")[:, None, None]
    gate = 1.0 / (1.0 + np.exp(-g2))
    pos = gate * t + (1.0 - gate) * f                      # [C, H, W]
    return np.ascontiguousarray(pos.reshape(C, N).T).astype(F32)  # [N, C]


def _host_sim():
    ii, jj = np.meshgrid(np.arange(HH, dtype=np.float64), np.arange(WW, dtype=np.float64), indexing="ij")
    pos2 = np.stack([ii.ravel(), jj.ravel()], axis=1)      # [N, 2]
    d2 = ((pos2[:, None, :] - pos2[None, :, :]) ** 2).sum(-1)
    return np.exp(-np.sqrt(d2)).astype(F32)                # [N, N]


def _host_idx():
    return np.linspace(0, N - 1, K).astype(F32).astype(np.int32)


# ------------------------------------------------------------ device program
_PROG = {}


def _build_program():
    import concourse.bacc as bacc
    import concourse.mybir as mybir
    import concourse.tile as tile
    from concourse.alu_op_type import AluOpType as op

    dt = mybir.dt
    AX = mybir.AxisListType.X
    ACT = mybir.ActivationFunctionType

    nc = bacc.Bacc("TRN2", target_bir_lowering=False, debug=False)

    dti = lambda name, shape: nc.dram_tensor(name, shape, dt.float32, kind="ExternalInput")
    xt_d = dti("xt", [C, N])
    x_d = dti("x", [N, C])
    post_d = dti("post", [C, N])
    pos_d = dti("pos", [N, C])
    sim_d = dti("sim", [N, N])
    c0t_d = dti("c0t", [C, K])
    iotab_d = dti("iotab", [128, K])
    eye_d = dti("eye", [128, 128])
    spw1_d = dti("spw1", [128, 4, C2])
    spb1_d = dti("spb1", [128, 2])
    spw2_d = dti("spw2", [128, 2])
    spb2_d = dti("spb2", [128, 1])
    fuw1_d = dti("fuw1", [128, 8, C2])
    fub1_d = dti("fub1", [128, 2])
    fuw2_d = dti("fuw2", [128, 2])
    fub2_d = dti("fub2", [1, 1])

    outv_d = nc.dram_tensor("outv", [1, C], dt.float32, kind="ExternalOutput")
    cent_d = nc.dram_tensor("centers", [K, C], dt.float32, kind="ExternalOutput")

    with tile.TileContext(nc) as tc:
        pers = tc.alloc_tile_pool(name="pers", bufs=1)
        XH1 = pers.tile([128, CT, N], dt.float16, tag="XH1")
        XH2 = pers.tile([128, CT, N], dt.float16, tag="XH2")
        EH1 = pers.tile([128, NT, C], dt.float16, tag="EH1")
        EH2 = pers.tile([128, NT, C], dt.float16, tag="EH2")
        GH1 = pers.tile([128, CT, K], dt.float16, tag="GH1")
        GH2 = pers.tile([128, CT, K], dt.float16, tag="GH2")
        OH = pers.tile([128, NT, K], dt.float16, tag="OH")
        ONES1H = pers.tile([128, 1], dt.float16, tag="ONES1H")
        V2H = pers.tile([128, NT, 4], dt.float16, tag="V2H")
        CENT = pers.tile([128, KT, C], dt.float32, tag="CENT")
        CENTT = pers.tile([128, CT, K], dt.float32, tag="CENTT")
        IOTB = pers.tile([128, K], dt.float32, tag="IOTB")
        EYE = pers.tile([128, 128], dt.float32, tag="EYE")
        LAB = pers.tile([128, NT], dt.float32, tag="LAB")
        SWT = pers.tile([128, NT], dt.float32, tag="SWT")
        WWT = pers.tile([128, NT], dt.float32, tag="WWT")
        WHIF = pers.tile([128, NT], dt.float32, tag="WHIF")
        WLOF = pers.tile([128, NT], dt.float32, tag="WLOF")
        CNTSB = pers.tile([1, K], dt.float32, tag="CNTSB")
        CWS = pers.tile([128, KT, 5], dt.float32, tag="CWS")
        SSWC = pers.tile([128, KT], dt.float32, tag="SSWC")
        ONES1 = pers.tile([128, 1], dt.float32, tag="ONES1")
        ONESR = pers.tile([1, 128], dt.float32, tag="ONESR")
        PSR = pers.tile([1, C], dt.float32, tag="PSR")      # possum row

        nc.vector.memset(ONES1[:], 1.0)
        nc.vector.memset(ONES1H[:], 1.0)
        nc.vector.memset(ONESR[:], 1.0)
        nc.vector.memset(CWS[:], 0.0)

        # ---- loads
        nc.sync.dma_start(CENTT[:], c0t_d.ap().rearrange("(a p) f -> p a f", p=128))
        nc.sync.dma_start(IOTB[:], iotab_d.ap())
        nc.sync.dma_start(EYE[:], eye_d.ap())

        # ---- prologue: enhanced = features + pos_emb, fp16 hi/lo splits, possum
        with tc.tile_pool(name="stream", bufs=2) as st, \
             tc.tile_pool(name="psr", bufs=1, space="PSUM") as psrp:
            for ct in range(CT):
                xtf = st.tile([128, N], dt.float32, tag="xtf")
                nc.sync.dma_start(xtf[:], xt_d.ap().rearrange("(a p) f -> p a f", p=128)[:, ct, :])
                pt = st.tile([128, N], dt.float32, tag="post")
                nc.sync.dma_start(pt[:], post_d.ap().rearrange("(a p) f -> p a f", p=128)[:, ct, :])
                nc.vector.tensor_tensor(xtf[:], xtf[:], pt[:], op.add)
                nc.vector.tensor_copy(XH1[:, ct, :], xtf[:])
                nc.scalar.copy(pt[:], XH1[:, ct, :])
                nc.vector.tensor_tensor(xtf[:], xtf[:], pt[:], op.subtract)
                nc.vector.tensor_copy(XH2[:, ct, :], xtf[:])
            psp = psrp.tile([1, C], dt.float32, tag="psp")
            for t in range(NT):
                xef = st.tile([128, C], dt.float32, tag="xef")
                nc.sync.dma_start(xef[:], x_d.ap().rearrange("(t p) f -> p t f", p=128)[:, t, :])
                pp = st.tile([128, C], dt.float32, tag="pos")
                nc.sync.dma_start(pp[:], pos_d.ap().rearrange("(t p) f -> p t f", p=128)[:, t, :])
                nc.tensor.matmul(psp[0:1, :], ONES1[:], pp[:], start=(t == 0), stop=(t == NT - 1))
                nc.vector.tensor_tensor(xef[:], xef[:], pp[:], op.add)
                nc.vector.tensor_copy(EH1[:, t, :], xef[:])
                nc.scalar.copy(pp[:], EH1[:, t, :])
                nc.vector.tensor_tensor(xef[:], xef[:], pp[:], op.subtract)
                nc.vector.tensor_copy(EH2[:, t, :], xef[:])
            nc.scalar.copy(PSR[:], psp[0:1, :])

        # ---- spatial MLP (fp16): relu1T = relu(spw1.T @ xeT), sw = sigmoid(...)
        with tc.tile_pool(name="r1", bufs=1) as r1p:
            R1T = r1p.tile([128, 2, N], dt.float16, tag="R1T")
            SPW1 = r1p.tile([128, 4, C2], dt.float32, tag="SPW1")
            SPW1H = r1p.tile([128, 4, C2], dt.float16, tag="SPW1H")
            SPB1 = r1p.tile([128, 2], dt.float32, tag="SPB1")
            SPW2 = r1p.tile([128, 2], dt.float32, tag="SPW2")
            SPW2H = r1p.tile([128, 2], dt.float16, tag="SPW2H")
            SPB2 = r1p.tile([128, 1], dt.float32, tag="SPB2")
            nc.sync.dma_start(SPW1[:], spw1_d.ap())
            nc.sync.dma_start(SPB1[:], spb1_d.ap())
            nc.sync.dma_start(SPW2[:], spw2_d.ap())
            nc.sync.dma_start(SPB2[:], spb2_d.ap())
            nc.vector.tensor_copy(SPW1H[:], SPW1[:])
            nc.vector.tensor_copy(SPW2H[:], SPW2[:])
            with tc.tile_pool(name="sp1", bufs=2, space="PSUM") as sp1:
                chunks = [(i * 512, min(512, N - i * 512)) for i in range((N + 511) // 512)]
                for m in range(2):
                    for off, w in chunks:
                        pc = sp1.tile([128, 512], dt.float32, tag="sp")
                        for kt in range(CT):
                            nc.tensor.matmul(pc[:, 0:w], SPW1H[:, kt, m * 128:(m + 1) * 128],
                                             XH1[:, kt, off:off + w],
                                             start=(kt == 0), stop=(kt == CT - 1))
                        nc.scalar.activation(R1T[:, m, off:off + w], pc[:, 0:w], ACT.Relu,
                                             bias=SPB1[:, m:m + 1])
            with tc.tile_pool(name="sp2", bufs=1, space="PSUM") as sp2:
                swp = sp2.tile([128, NT], dt.float32, tag="swp")
                for t in range(NT):
                    for q in range(2):
                        nc.tensor.matmul(swp[:, t:t + 1], R1T[:, q, t * 128:(t + 1) * 128],
                                         SPW2H[:, q:q + 1], start=(q == 0), stop=(q == 1))
                nc.scalar.activation(SWT[:], swp[:], ACT.Sigmoid, bias=SPB2[:, 0:1])
                nc.vector.tensor_copy(V2H[:, :, 2], SWT[:])
                nc.scalar.copy(WHIF[:], V2H[:, :, 2])
                nc.vector.tensor_tensor(WLOF[:], SWT[:], WHIF[:], op.subtract)
                nc.vector.tensor_copy(V2H[:, :, 3], WLOF[:])

        # ---- kmeans: 10 update iterations + final assignment pass
        with tc.tile_pool(name="small", bufs=4) as small, \
             tc.tile_pool(name="medium", bufs=2) as medium:
            for it in range(ITERS + 1):
                itb = tc.alloc_tile_pool(name=f"itb{it}", bufs=1)
                CCB = itb.tile([128, K], dt.float32, tag="CCB")
                CCH = itb.tile([1, K], dt.float32, tag="CCH")
                # fp16 hi/lo split of current centroids
                for ct in range(CT):
                    gt = medium.tile([128, K], dt.float32, tag="gt")
                    nc.vector.tensor_copy(GH1[:, ct, :], CENTT[:, ct, :])
                    nc.scalar.copy(gt[:], GH1[:, ct, :])
                    nc.vector.tensor_tensor(gt[:], CENTT[:, ct, :], gt[:], op.subtract)
                    nc.vector.tensor_copy(GH2[:, ct, :], gt[:])
                # ccb = broadcast of 0.5*colsum(centT^2) [128, K]
                with tc.tile_pool(name=f"cc{it}", bufs=1, space="PSUM") as ccp_p, \
                     tc.tile_pool(name=f"sq{it}", bufs=2) as sqp:
                    ccp = ccp_p.tile([1, K], dt.float32, tag="ccp")
                    for ct in range(CT):
                        sq = sqp.tile([128, K], dt.float32, tag="sq")
                        nc.scalar.activation(sq[:], CENTT[:, ct, :], ACT.Square)
                        nc.tensor.matmul(ccp[0:1, 0:512], ONES1[:], sq[:, 0:512],
                                         start=(ct == 0), stop=(ct == CT - 1))
                        nc.tensor.matmul(ccp[0:1, 512:K], ONES1[:], sq[:, 512:K],
                                         start=(ct == 0), stop=(ct == CT - 1))
                    nc.scalar.activation(CCH[:], ccp[0:1, :], ACT.Copy, scale=0.5)
                with tc.tile_pool(name=f"cb{it}", bufs=1, space="PSUM") as cbp:
                    pcb = cbp.tile([128, 1024], dt.float32, tag="pcb")
                    nc.tensor.matmul(pcb[:, 0:512], ONESR[:], CCH[0:1, 0:512], start=True, stop=True)
                    nc.tensor.matmul(pcb[:, 512:K], ONESR[:], CCH[0:1, 512:K], start=True, stop=True)
                    nc.scalar.copy(CCB[:], pcb[:, 0:K])

                # distances (fp16 3-term) + argmax(S - cc/2) -> labels, one-hot, counts
                with tc.tile_pool(name=f"d{it}", bufs=3, space="PSUM") as dps, \
                     tc.tile_pool(name=f"cn{it}", bufs=1, space="PSUM") as cnp:
                    pcnt = cnp.tile([1, K], dt.float32, tag="pcnt")
                    for t in range(NT):
                        S = dps.tile([128, K], dt.float32, tag="S")
                        tb = slice(t * 128, (t + 1) * 128)
                        pairs = [(XH1, GH1), (XH1, GH2), (XH2, GH1)]
                        for ct in range(CT):
                            for pi, (xh, gh) in enumerate(pairs):
                                st_ = (ct == 0 and pi == 0)
                                sp_ = (ct == CT - 1 and pi == 2)
                                nc.tensor.matmul(S[:, 0:512], xh[:, ct, tb], gh[:, ct, 0:512],
                                                 start=st_, stop=sp_)
                                nc.tensor.matmul(S[:, 512:K], xh[:, ct, tb], gh[:, ct, 512:K],
                                                 start=st_, stop=sp_)
                        dm = medium.tile([128, K], dt.float32, tag="dm")
                        nc.vector.tensor_tensor(dm[:], S[:, 0:K], CCB[:], op.subtract)
                        mx = small.tile([128, 8], dt.float32, tag="mx")
                        mi = small.tile([128, 8], dt.uint32, tag="mi")
                        nc.vector.max(mx[:], dm[:])
                        nc.vector.max_index(mi[:], mx[:], dm[:])
                        nc.vector.tensor_copy(LAB[:, t:t + 1], mi[:, 0:1])
                        nc.vector.tensor_scalar(OH[:, t, :], IOTB[:], LAB[:, t:t + 1], None, op.is_equal)
                        nc.tensor.matmul(pcnt[0:1, 0:512], ONES1H[:], OH[:, t, 0:512],
                                         start=(t == 0), stop=(t == NT - 1))
                        nc.tensor.matmul(pcnt[0:1, 512:K], ONES1H[:], OH[:, t, 512:K],
                                         start=(t == 0), stop=(t == NT - 1))
                    nc.scalar.copy(CNTSB[:], pcnt[0:1, :])

                if it == ITERS:
                    itb.release()
                    break

                # broadcast counts, masks
                CNTB = itb.tile([128, K], dt.float32, tag="CNTB")
                CMB = itb.tile([128, K], dt.float32, tag="CMB")
                MGB = itb.tile([128, K], dt.uint8, tag="MGB")
                with tc.tile_pool(name=f"bc{it}", bufs=1, space="PSUM") as bcp:
                    pb = bcp.tile([128, 1024], dt.float32, tag="pb")
                    nc.tensor.matmul(pb[:, 0:512], ONESR[:], CNTSB[0:1, 0:512], start=True, stop=True)
                    nc.tensor.matmul(pb[:, 512:K], ONESR[:], CNTSB[0:1, 512:K], start=True, stop=True)
                    nc.scalar.copy(CNTB[:], pb[:, 0:K])
                nc.vector.tensor_scalar(CMB[:], CNTB[:], 1.0, None, op.max)
                nc.vector.reciprocal(CMB[:], CMB[:])
                nc.vector.tensor_scalar(MGB[:], CNTB[:], 0.0, None, op.is_gt)

                # centroid sums in transposed layout; update CENTT in place
                with tc.tile_pool(name=f"s{it}", bufs=1, space="PSUM") as spsp:
                    PST = spsp.tile([128, CT, 1024], dt.float32, tag="PST")
                    for t in range(NT):
                        for ct in range(CT):
                            cb = slice(ct * 128, (ct + 1) * 128)
                            for ei, eh in enumerate((EH1, EH2)):
                                st_ = (t == 0 and ei == 0)
                                sp_ = (t == NT - 1 and ei == 1)
                                nc.tensor.matmul(PST[:, ct, 0:512], eh[:, t, cb],
                                                 OH[:, t, 0:512], start=st_, stop=sp_)
                                nc.tensor.matmul(PST[:, ct, 512:K], eh[:, t, cb],
                                                 OH[:, t, 512:K], start=st_, stop=sp_)
                    for ct in range(CT):
                        q = medium.tile([128, K], dt.float32, tag="q")
                        nc.vector.tensor_tensor(q[:], PST[:, ct, 0:K], CMB[:], op.mult)
                        nc.vector.copy_predicated(CENTT[:, ct, :], MGB[:], q[:])
                itb.release()

            # ---- build CENT [kk, C] from final CENTT
            with tc.tile_pool(name="tr0", bufs=2, space="PSUM") as trp:
                for r in range(KT):
                    rows = KROWS[r]
                    for ct in range(CT):
                        tp = trp.tile([128, 128], dt.float32, tag="tp")
                        nc.tensor.transpose(tp[0:rows, 0:128], CENTT[:, ct, r * 128:r * 128 + rows],
                                            EYE[:, :])
                        nc.scalar.copy(CENT[0:rows, r, ct * 128:(ct + 1) * 128], tp[0:rows, 0:128])

            # ---- refine
            with tc.tile_pool(name="lrow", bufs=1) as lrp:
                LROW = lrp.tile([1, N], dt.float32, tag="LROW")
                LROWB = lrp.tile([128, N], dt.float32, tag="LROWB")
                for t in range(NT):
                    nc.sync.dma_start(LROW[0:1, t * 128:(t + 1) * 128], LAB[:, t:t + 1])
                with tc.tile_pool(name="pbl", bufs=2, space="PSUM") as pbp:
                    for off, w in [(i * 512, min(512, N - i * 512)) for i in range((N + 511) // 512)]:
                        pb2 = pbp.tile([128, 512], dt.float32, tag="pb2")
                        nc.tensor.matmul(pb2[:, 0:w], ONESR[:], LROW[0:1, off:off + w], start=True, stop=True)
                        nc.scalar.copy(LROWB[:, off:off + w], pb2[:, 0:w])

                # per-point weights w_n = sum_{m in cluster(n)} sim[n, m]
                with tc.tile_pool(name="simst", bufs=2) as sst:
                    for t in range(NT):
                        smt = sst.tile([128, N], dt.float32, tag="simt")
                        nc.sync.dma_start(smt[:], sim_d.ap()[t * 128:(t + 1) * 128, :])
                        nc.vector.scalar_tensor_tensor(smt[:], LROWB[:], LAB[:, t:t + 1], smt[:],
                                                       op.is_equal, op.mult,
                                                       accum_out=WWT[:, t:t + 1])

            with tc.tile_pool(name="ref", bufs=1) as refp:
                # fp16 hi/lo split of per-point weights w into V2H cols 0,1
                AUXSB = refp.tile([4, K], dt.float32, tag="AUXSB")
                nc.vector.tensor_copy(V2H[:, :, 0], WWT[:])
                nc.scalar.copy(WHIF[:], V2H[:, :, 0])
                nc.vector.tensor_tensor(WLOF[:], WWT[:], WHIF[:], op.subtract)
                nc.vector.tensor_copy(V2H[:, :, 1], WLOF[:])

                # pass a: aux rows [whi; wlo; swhi; swlo].T @ oh
                with tc.tile_pool(name="rsa", bufs=1, space="PSUM") as rpsa:
                    AUX = rpsa.tile([4, K], dt.float32, tag="AUX")
                    for t in range(NT):
                        st_, sp_ = (t == 0), (t == NT - 1)
                        nc.tensor.matmul(AUX[0:4, 0:512], V2H[:, t, :], OH[:, t, 0:512],
                                         start=st_, stop=sp_)
                        nc.tensor.matmul(AUX[0:4, 512:K], V2H[:, t, :], OH[:, t, 512:K],
                                         start=st_, stop=sp_)
                    nc.scalar.copy(AUXSB[:], AUX[0:4, :])

                # transpose aux/cnt rows into per-kk columns
                with tc.tile_pool(name="tr1", bufs=2, space="PSUM") as tr1:
                    for r in range(KT):
                        rows = KROWS[r]
                        sl = slice(r * 128, r * 128 + rows)
                        tpa = tr1.tile([128, 4], dt.float32, tag="tpa")
                        nc.tensor.transpose(tpa[0:rows, 0:4], AUXSB[0:4, sl], EYE[0:4, 0:4])
                        nc.scalar.copy(CWS[0:rows, r, 0:4], tpa[0:rows, 0:4])
                        tpc = tr1.tile([128, 1], dt.float32, tag="tpc")
                        nc.tensor.transpose(tpc[0:rows, 0:1], CNTSB[0:1, sl], EYE[0:1, 0:1])
                        nc.scalar.copy(CWS[0:rows, r, 4:5], tpc[0:rows, 0:1])
                nc.vector.tensor_tensor(SSWC[:], CWS[:, :, 2], CWS[:, :, 3], op.add)

                # pass b: weighted sums (kk layout, fp16 splits) + blend
                with tc.tile_pool(name="rsb", bufs=1, space="PSUM") as rpsb:
                    RSUM = rpsb.tile([128, KT, C], dt.float32, tag="RSUM")
                    with tc.tile_pool(name="roh", bufs=3) as rohp:
                        for t in range(NT):
                            ohwh = rohp.tile([128, K], dt.float16, tag="rohwh")
                            ohwl = rohp.tile([128, K], dt.float16, tag="rohwl")
                            nc.vector.tensor_scalar(ohwh[:], IOTB[:], LAB[:, t:t + 1], WHIF[:, t:t + 1],
                                                    op.is_equal, op.mult)
                            nc.vector.tensor_scalar(ohwl[:], IOTB[:], LAB[:, t:t + 1], WLOF[:, t:t + 1],
                                                    op.is_equal, op.mult)
                            st_, sp_ = (t == 0), (t == NT - 1)
                            terms = [(ohwh, EH1), (ohwh, EH2), (ohwl, EH1)]
                            for r in range(KT):
                                rows = KROWS[r]
                                for pi, (ow, eh) in enumerate(terms):
                                    nc.tensor.matmul(RSUM[0:rows, r, :], ow[:, r * 128:r * 128 + rows],
                                                     eh[:, t, :], start=(st_ and pi == 0),
                                                     stop=(sp_ and pi == 2))
                    # centers = where(cnt>0, rsum/(wsum + max(cnt,1)*1e-6), cent)
                    for r in range(KT):
                        rows = KROWS[r]
                        cm = small.tile([128, 1], dt.float32, tag="cm")
                        ws2 = small.tile([128, 1], dt.float32, tag="ws2")
                        wsp = small.tile([128, 1], dt.float32, tag="wsp")
                        mg = small.tile([128, 1], dt.float32, tag="mg")
                        dd = medium.tile([128, C], dt.float32, tag="dd")
                        nc.vector.tensor_scalar(cm[0:rows], CWS[0:rows, r, 4:5], 1.0, None, op.max)
                        nc.vector.tensor_tensor(ws2[0:rows], CWS[0:rows, r, 0:1], CWS[0:rows, r, 1:2], op.add)
                        nc.vector.scalar_tensor_tensor(wsp[0:rows], cm[0:rows], 1e-6,
                                                       ws2[0:rows, 0:1], op.mult, op.add)
                        nc.vector.reciprocal(wsp[0:rows], wsp[0:rows])
                        nc.vector.tensor_scalar(mg[0:rows], CWS[0:rows, r, 4:5], 0.0, None, op.is_gt)
                        nc.vector.scalar_tensor_tensor(dd[0:rows, :], RSUM[0:rows, r, :], wsp[0:rows, 0:1],
                                                       CENT[0:rows, r, :], op.mult, op.subtract)
                        nc.vector.scalar_tensor_tensor(CENT[0:rows, r, :], dd[0:rows, :], mg[0:rows, 0:1],
                                                       CENT[0:rows, r, :], op.mult, op.add)
                        nc.sync.dma_start(cent_d.ap()[r * 128:r * 128 + rows, :], CENT[0:rows, r, :])

            # ---- epilogue: means + fusion gate (row layout)
            with tc.tile_pool(name="epb", bufs=1) as epb, \
                 tc.tile_pool(name="ep", bufs=1, space="PSUM") as eps:
                FUW1 = epb.tile([128, 8, C2], dt.float32, tag="FUW1")
                FUB1 = epb.tile([128, 2], dt.float32, tag="FUB1")
                FUW2 = epb.tile([128, 2], dt.float32, tag="FUW2")
                FUB2 = epb.tile([1, 1], dt.float32, tag="FUB2")
                nc.sync.dma_start(FUW1[:], fuw1_d.ap())
                nc.sync.dma_start(FUB1[:], fub1_d.ap())
                nc.sync.dma_start(FUW2[:], fuw2_d.ap())
                nc.sync.dma_start(FUB2[:], fub2_d.ap())
                CER = epb.tile([1, C], dt.float32, tag="CER")
                GROW = epb.tile([1, C], dt.float32, tag="GROW")
                LROW_ = epb.tile([1, C], dt.float32, tag="LROW_")
                T2R = epb.tile([1, C], dt.float32, tag="T2R")
                T3R = epb.tile([1, C], dt.float32, tag="T3R")
                OROW = epb.tile([1, C], dt.float32, tag="OROW")
                GBUF = epb.tile([128, 8], dt.float32, tag="GBUF")
                H2 = epb.tile([128, 2], dt.float32, tag="H2")
                ASB = epb.tile([1, 1], dt.float32, tag="ASB")
                dgl = epb.tile([1, C], dt.float32, tag="dgl")
                cer_p = eps.tile([1, C], dt.float32, tag="cer_p")
                t2_p = eps.tile([1, C], dt.float32, tag="t2_p")
                t3_p = eps.tile([1, C], dt.float32, tag="t3_p")
                for t in range(NT):
                    for ei, eh in enumerate((EH1, EH2)):
                        nc.tensor.matmul(cer_p[0:1, :], ONES1H[:], eh[:, t, :],
                                         start=(t == 0 and ei == 0), stop=(t == NT - 1 and ei == 1))
                    t2terms = [(2, EH1), (2, EH2), (3, EH1)]
                    for pi, (col, eh) in enumerate(t2terms):
                        nc.tensor.matmul(t2_p[0:1, :], V2H[:, t, col:col + 1], eh[:, t, :],
                                         start=(t == 0 and pi == 0), stop=(t == NT - 1 and pi == 2))
                for r in range(KT):
                    rows = KROWS[r]
                    nc.tensor.matmul(t3_p[0:1, :], SSWC[0:rows, r:r + 1], CENT[0:rows, r, :],
                                     start=(r == 0), stop=(r == KT - 1))
                nc.scalar.copy(CER[:], cer_p[0:1, :])
                nc.scalar.copy(T2R[:], t2_p[0:1, :])
                nc.scalar.copy(T3R[:], t3_p[0:1, :])
                # global = (cer - psr)/N ; local = (cer + t3 - t2)/N
                nc.vector.tensor_tensor(GROW[:], CER[:], PSR[:], op.subtract)
                nc.vector.tensor_scalar(GROW[:], GROW[:], 1.0 / N, None, op.mult)
                nc.vector.tensor_tensor(LROW_[:], CER[:], T3R[:], op.add)
                nc.vector.tensor_tensor(LROW_[:], LROW_[:], T2R[:], op.subtract)
                nc.vector.tensor_scalar(LROW_[:], LROW_[:], 1.0 / N, None, op.mult)

                # transpose [g | l] rows into GBUF columns (K-layout for fusion)
                with tc.tile_pool(name="tr2", bufs=2, space="PSUM") as tr2:
                    for j in range(8):
                        src = GROW if j < 4 else LROW_
                        o = (j % 4) * 128
                        tpg = tr2.tile([128, 1], dt.float32, tag="tpg")
                        nc.tensor.transpose(tpg[0:128, 0:1], src[0:1, o:o + 128], EYE[0:1, 0:1])
                        nc.scalar.copy(GBUF[:, j:j + 1], tpg[:, 0:1])

                pfu = eps.tile([128, 2, 512], dt.float32, tag="pfu")
                pa = eps.tile([1, 1], dt.float32, tag="pa")
                for m in range(2):
                    for kt in range(8):
                        nc.tensor.matmul(pfu[:, m, 0:1], FUW1[:, kt, m * 128:(m + 1) * 128],
                                         GBUF[:, kt:kt + 1], start=(kt == 0), stop=(kt == 7))
                    nc.scalar.activation(H2[:, m:m + 1], pfu[:, m, 0:1], ACT.Relu, bias=FUB1[:, m:m + 1])
                for m in range(2):
                    nc.tensor.matmul(pa[0:1, 0:1], H2[:, m:m + 1], FUW2[:, m:m + 1],
                                     start=(m == 0), stop=(m == 1))
                nc.scalar.activation(ASB[:], pa[0:1, 0:1], ACT.Sigmoid, bias=FUB2[0:1, 0:1])
                # out = l + a*(g - l)
                nc.vector.tensor_tensor(dgl[:], GROW[:], LROW_[:], op.subtract)
                nc.vector.scalar_tensor_tensor(OROW[:], dgl[:], ASB[0:1, 0:1], LROW_[:], op.mult, op.add)
                nc.sync.dma_start(outv_d.ap(), OROW[:])

        pers.release()

    nc.compile()
    return nc


def _get_program():
    if "nc" not in _PROG:
        _PROG["nc"] = _build_program()
    return _PROG["nc"]


# ------------------------------------------------------------------- kernel
def _prep_in_maps(features, params):
    feats = np.asarray(features, F32)
    p = {k: np.asarray(v, F32) for k, v in params.items()}

    pos = _host_pos_emb(p)                       # [N, C] f32
    post = np.ascontiguousarray(pos.T)           # [C, N]
    sim = _host_sim()                            # [N, N]
    idx = _host_idx()                            # [K]
    iotab = np.broadcast_to(np.arange(K, dtype=F32), (128, K)).copy()
    eye = np.eye(128, dtype=F32)

    spw1 = np.ascontiguousarray(p["sp_w1"].reshape(4, 128, C2).transpose(1, 0, 2))
    spb1 = np.ascontiguousarray(p["sp_b1"].reshape(2, 128).T)
    spw2 = np.ascontiguousarray(p["sp_w2"].reshape(2, 128).T)
    spb2 = np.full((128, 1), p["sp_b2"][0], F32)
    fuw1 = np.ascontiguousarray(p["fu_w1"].reshape(8, 128, C2).transpose(1, 0, 2))
    fub1 = np.ascontiguousarray(p["fu_b1"].reshape(2, 128).T)
    fuw2 = np.ascontiguousarray(p["fu_w2"].reshape(2, 128).T)
    fub2 = np.full((1, 1), p["fu_b2"][0], F32)

    shared = dict(post=post, pos=pos, sim=sim, iotab=iotab, eye=eye,
                  spw1=spw1, spb1=spb1, spw2=spw2, spb2=spb2,
                  fuw1=fuw1, fub1=fub1, fuw2=fuw2, fub2=fub2)

    in_maps = []
    for b in range(B):
        xt = np.ascontiguousarray(feats[b].reshape(C, N))
        x = np.ascontiguousarray(xt.T)
        xe = x + pos
        c0t = np.ascontiguousarray(xe[idx].T)
        in_maps.append(dict(xt=xt, x=x, c0t=c0t, **shared))
    return in_maps


def kernel(features, params):
    from concourse import bass_utils

    nc = _get_program()
    in_maps = _prep_in_maps(features, params)
    res = bass_utils.run_bass_kernel_spmd(nc, in_maps, core_ids=list(range(B)))
    if res.exec_time_ns is not None:
        print(f"HW exec time: {res.exec_time_ns} ns")

    out = np.empty((B, C), F32)
    centers = np.empty((B, K, C), F32)
    for b, r in enumerate(res.results):
        out[b] = r["outv"].reshape(C)
        centers[b] = r["centers"]
    return out, centers


# revision 34
# speedup vs baseline: 1.8833x; 1.0047x over previous
"""Trainium2 Bass kernel for nn_LocalFeatureCluster (vq_codebook).

Data-parallel over batch: each of the 8 NeuronCores runs one sample's full
kmeans + sim-weighted refine + MLP pipeline. Host prepares constants
(sim matrix, init-centroid gather, tiny position-encoder) and shards.
"""
import math
import os
import sys
import types

import numpy as np

# ---------------------------------------------------------------- constants
B, C, HH, WW = 8, 512, 48, 48
N = HH * WW            # 2304
K = 691                # max(2, int(N * 0.3))
C2 = C // 2            # 256
ITERS = 10
NT = N // 128          # 18 n-tiles
CT = C // 128          # 4 c-tiles
KT = 6                 # k-tiles: 5*128 + 51
KROWS = [128, 128, 128, 128, 128, 51]
F32 = np.float32

# ------------------------------------------------------- axon NTFF trace shim
def _install_trace_shim():
    if "antenv.axon_hooks" in sys.modules:
        return
    try:
        mod = types.ModuleType("antenv.axon_hooks")
        mod._hook = None

        def _set(h):
            mod._hook = h

        def _get():
            return mod._hook

        mod.set_axon_ntff_profile_hook = _set
        mod.get_axon_ntff_profile_hook = _get
        sys.modules["antenv.axon_hooks"] = mod
        import antenv

        antenv.axon_hooks = mod
        from trn_agent_boot.trn_boot import _ntff_profile_via_ctypes

        _set(_ntff_profile_via_ctypes("/opt/axon/libaxon_pjrt.so"))
    except Exception:
        pass


_install_trace_shim()

# ------------------------------------------------------------- host compute
def _host_pos_emb(p):
    """EnhancedPositionEncoder forward in float64 -> [N, C] float32."""
    from scipy.special import erf

    ls = np.linspace(-1.0, 1.0, HH).astype(np.float32).astype(np.float64)
    gx = np.broadcast_to(ls[None, :], (HH, WW)).astype(np.float64)  # varies with j
    gy = np.broadcast_to(ls[:, None], (HH, WW)).astype(np.float64)  # varies with i

    def gelu(x):
        return x * 0.5 * (1.0 + erf(x / np.sqrt(2.0)))

    def conv_h(x, w, b):
        # x [ci, H, W], w [co, ci, 3, 1], pad (1,0) along H
        xp = np.pad(x, ((0, 0), (1, 1), (0, 0)))
        col = np.stack([xp[:, k:k + HH, :] for k in range(3)], axis=1)
        y = np.einsum("cihw,oci->ohw", col, w[:, :, :, 0])
        return y + b[:, None, None]

    def conv_w(x, w, b):
        xp = np.pad(x, ((0, 0), (0, 0), (1, 1)))
        col = np.stack([xp[:, :, k:k + WW] for k in range(3)], axis=1)
        y = np.einsum("cihw,oci->ohw", col, w[:, :, 0, :])
        return y + b[:, None, None]

    g = lambda name: np.asarray(p[name], np.float64)
    t = conv_h(gx[None], g("te_w1"), g("te_b1"))
    t = conv_h(gelu(t), g("te_w2"), g("te_b2"))            # [C, H, W]
    f = conv_w(gy[None], g("fe_w1"), g("fe_b1"))
    f = conv_w(gelu(f), g("fe_w2"), g("fe_b2"))            # [C, H, W]
    comb = np.concatenate([t, f], axis=0)                  # [2C, H, W]
    g1 = np.einsum("chw,oc->ohw", comb, g("g_w1")[:, :, 0, 0]) + g("g_b1")[:, None, None]
    g1 = np.maximum(g1, 0.0)
    g2 = np.einsum("chw,oc->ohw", g1, g("g_w2")[:, :, 0, 0]) + g("g_b2")[:, None, None]
    gate = 1.0 / (1.0 + np.exp(-g2))
    pos = gate * t + (1.0 - gate) * f                      # [C, H, W]
    return np.ascontiguousarray(pos.reshape(C, N).T).astype(F32)  # [N, C]


def _host_sim():
    ii, jj = np.meshgrid(np.arange(HH, dtype=np.float64), np.arange(WW, dtype=np.float64), indexing="ij")
    pos2 = np.stack([ii.ravel(), jj.ravel()], axis=1)      # [N, 2]
    d2 = ((pos2[:, None, :] - pos2[None, :, :]) ** 2).sum(-1)
    return np.exp(-np.sqrt(d2)).astype(F32)                # [N, N]


def _host_idx():
    return np.linspace(0, N - 1, K).astype(F32).astype(np.int32)


# ------------------------------------------------------------ device program
_PROG = {}


def _build_program():
    import concourse.bacc as bacc
    import concourse.mybir as mybir
    import concourse.tile as tile
    from concourse.alu_op_type import AluOpType as op

    dt = mybir.dt
    AX = mybir.AxisListType.X
    ACT = mybir.ActivationFunctionType

    nc = bacc.Bacc("TRN2", target_bir_lowering=False, debug=False)

    dti = lambda name, shape: nc.dram_tensor(name, shape, dt.float32, kind="ExternalInput")
    xt_d = dti("xt", [C, N])
    x_d = dti("x", [N, C])
    post_d = dti("post", [C, N])
    pos_d = dti("pos", [N, C])
    sim_d = dti("sim", [N, N])
    c0t_d = dti("c0t", [C, K])
    iotab_d = dti("iotab", [128, K])
    eye_d = dti("eye", [128, 128])
    spw1_d = dti("spw1", [128, 4, C2])
    spb1_d = dti("spb1", [128, 2])
    spw2_d = dti("spw2", [128, 2])
    spb2_d = dti("spb2", [128, 1])
    fuw1_d = dti("fuw1", [128, 8, C2])
    fub1_d = dti("fub1", [128, 2])
    fuw2_d = dti("fuw2", [128, 2])
    fub2_d = dti("fub2", [1, 1])

    outv_d = nc.dram_tensor("outv", [1, C], dt.float32, kind="ExternalOutput")
    cent_d = nc.dram_tensor("centers", [K, C], dt.float32, kind="ExternalOutput")

    with tile.TileContext(nc) as tc:
        pers = tc.alloc_tile_pool(name="pers", bufs=1)
        XH1 = pers.tile([128, CT, N], dt.float16, tag="XH1")
        XH2 = pers.tile([128, CT, N], dt.float16, tag="XH2")
        EH1 = pers.tile([128, NT, C], dt.float16, tag="EH1")
        EH2 = pers.tile([128, NT, C], dt.float16, tag="EH2")
        GH1 = pers.tile([128, CT, K], dt.float16, tag="GH1")
        GH2 = pers.tile([128, CT, K], dt.float16, tag="GH2")
        OH = pers.tile([128, NT, K], dt.float16, tag="OH")
        ONES1H = pers.tile([128, 1], dt.float16, tag="ONES1H")
        V2H = pers.tile([128, NT, 4], dt.float16, tag="V2H")
        CENT = pers.tile([128, KT, C], dt.float32, tag="CENT")
        CENTT = pers.tile([128, CT, K], dt.float32, tag="CENTT")
        IOTB = pers.tile([128, K], dt.float32, tag="IOTB")
        EYE = pers.tile([128, 128], dt.float32, tag="EYE")
        LAB = pers.tile([128, NT], dt.float32, tag="LAB")
        SWT = pers.tile([128, NT], dt.float32, tag="SWT")
        WWT = pers.tile([128, NT], dt.float32, tag="WWT")
        WHIF = pers.tile([128, NT], dt.float32, tag="WHIF")
        WLOF = pers.tile([128, NT], dt.float32, tag="WLOF")
        CNTSB = pers.tile([1, K], dt.float32, tag="CNTSB")
        CWS = pers.tile([128, KT, 5], dt.float32, tag="CWS")
        SSWC = pers.tile([128, KT], dt.float32, tag="SSWC")
        ONES1 = pers.tile([128, 1], dt.float32, tag="ONES1")
        ONESR = pers.tile([1, 128], dt.float32, tag="ONESR")
        PSR = pers.tile([1, C], dt.float32, tag="PSR")      # possum row

        nc.vector.memset(ONES1[:], 1.0)
        nc.vector.memset(ONES1H[:], 1.0)
        nc.vector.memset(ONESR[:], 1.0)
        nc.vector.memset(CWS[:], 0.0)

        # ---- loads
        nc.sync.dma_start(CENTT[:], c0t_d.ap().rearrange("(a p) f -> p a f", p=128))
        nc.sync.dma_start(IOTB[:], iotab_d.ap())
        nc.sync.dma_start(EYE[:], eye_d.ap())

        # ---- prologue: enhanced = features + pos_emb, fp16 hi/lo splits, possum
        with tc.tile_pool(name="stream", bufs=2) as st, \
             tc.tile_pool(name="psr", bufs=1, space="PSUM") as psrp:
            for ct in range(CT):
                xtf = st.tile([128, N], dt.float32, tag="xtf")
                nc.sync.dma_start(xtf[:], xt_d.ap().rearrange("(a p) f -> p a f", p=128)[:, ct, :])
                pt = st.tile([128, N], dt.float32, tag="post")
                nc.sync.dma_start(pt[:], post_d.ap().rearrange("(a p) f -> p a f", p=128)[:, ct, :])
                nc.vector.tensor_tensor(xtf[:], xtf[:], pt[:], op.add)
                nc.vector.tensor_copy(XH1[:, ct, :], xtf[:])
                nc.scalar.copy(pt[:], XH1[:, ct, :])
                nc.vector.tensor_tensor(xtf[:], xtf[:], pt[:], op.subtract)
                nc.vector.tensor_copy(XH2[:, ct, :], xtf[:])
            psp = psrp.tile([1, C], dt.float32, tag="psp")
            for t in range(NT):
                xef = st.tile([128, C], dt.float32, tag="xef")
                nc.sync.dma_start(xef[:], x_d.ap().rearrange("(t p) f -> p t f", p=128)[:, t, :])
                pp = st.tile([128, C], dt.float32, tag="pos")
                nc.sync.dma_start(pp[:], pos_d.ap().rearrange("(t p) f -> p t f", p=128)[:, t, :])
                nc.tensor.matmul(psp[0:1, :], ONES1[:], pp[:], start=(t == 0), stop=(t == NT - 1))
                nc.vector.tensor_tensor(xef[:], xef[:], pp[:], op.add)
                nc.vector.tensor_copy(EH1[:, t, :], xef[:])
                nc.scalar.copy(pp[:], EH1[:, t, :])
                nc.vector.tensor_tensor(xef[:], xef[:], pp[:], op.subtract)
                nc.vector.tensor_copy(EH2[:, t, :], xef[:])
            nc.scalar.copy(PSR[:], psp[0:1, :])

        # ---- spatial MLP (fp16): relu1T = relu(spw1.T @ xeT), sw = sigmoid(...)
        with tc.tile_pool(name="r1", bufs=1) as r1p:
            R1T = r1p.tile([128, 2, N], dt.float16, tag="R1T")
            SPW1 = r1p.tile([128, 4, C2], dt.float32, tag="SPW1")
            SPW1H = r1p.tile([128, 4, C2], dt.float16, tag="SPW1H")
            SPB1 = r1p.tile([128, 2], dt.float32, tag="SPB1")
            SPW2 = r1p.tile([128, 2], dt.float32, tag="SPW2")
            SPW2H = r1p.tile([128, 2], dt.float16, tag="SPW2H")
            SPB2 = r1p.tile([128, 1], dt.float32, tag="SPB2")
            nc.sync.dma_start(SPW1[:], spw1_d.ap())
            nc.sync.dma_start(SPB1[:], spb1_d.ap())
            nc.sync.dma_start(SPW2[:], spw2_d.ap())
            nc.sync.dma_start(SPB2[:], spb2_d.ap())
            nc.vector.tensor_copy(SPW1H[:], SPW1[:])
            nc.vector.tensor_copy(SPW2H[:], SPW2[:])
            with tc.tile_pool(name="sp1", bufs=2, space="PSUM") as sp1:
                chunks = [(i * 512, min(512, N - i * 512)) for i in range((N + 511) // 512)]
                for m in range(2):
                    for off, w in chunks:
                        pc = sp1.tile([128, 512], dt.float32, tag="sp")
                        for kt in range(CT):
                            nc.tensor.matmul(pc[:, 0:w], SPW1H[:, kt, m * 128:(m + 1) * 128],
                                             XH1[:, kt, off:off + w],
                                             start=(kt == 0), stop=(kt == CT - 1))
                        nc.scalar.activation(R1T[:, m, off:off + w], pc[:, 0:w], ACT.Relu,
                                             bias=SPB1[:, m:m + 1])
            with tc.tile_pool(name="sp2", bufs=1, space="PSUM") as sp2:
                swp = sp2.tile([128, NT], dt.float32, tag="swp")
                for t in range(NT):
                    for q in range(2):
                        nc.tensor.matmul(swp[:, t:t + 1], R1T[:, q, t * 128:(t + 1) * 128],
                                         SPW2H[:, q:q + 1], start=(q == 0), stop=(q == 1))
                nc.scalar.activation(SWT[:], swp[:], ACT.Sigmoid, bias=SPB2[:, 0:1])
                nc.vector.tensor_copy(V2H[:, :, 2], SWT[:])
                nc.scalar.copy(WHIF[:], V2H[:, :, 2])
                nc.vector.tensor_tensor(WLOF[:], SWT[:], WHIF[:], op.subtract)
                nc.vector.tensor_copy(V2H[:, :, 3], WLOF[:])

        # ---- kmeans: 10 update iterations + final assignment pass
        with tc.tile_pool(name="small", bufs=4) as small, \
             tc.tile_pool(name="medium", bufs=2) as medium:
            for it in range(ITERS + 1):
                itb = tc.alloc_tile_pool(name=f"itb{it}", bufs=1)
                CCB = itb.tile([128, K], dt.float32, tag="CCB")
                CCH = itb.tile([1, K], dt.float32, tag="CCH")
                # fp16 hi/lo split of current centroids
                for ct in range(CT):
                    gt = medium.tile([128, K], dt.float32, tag="gt")
                    nc.vector.tensor_copy(GH1[:, ct, :], CENTT[:, ct, :])
                    nc.scalar.copy(gt[:], GH1[:, ct, :])
                    nc.vector.tensor_tensor(gt[:], CENTT[:, ct, :], gt[:], op.subtract)
                    nc.vector.tensor_copy(GH2[:, ct, :], gt[:])
                # ccb = broadcast of 0.5*colsum(centT^2) [128, K]
                with tc.tile_pool(name=f"cc{it}", bufs=1, space="PSUM") as ccp_p, \
                     tc.tile_pool(name=f"sq{it}", bufs=2) as sqp:
                    ccp = ccp_p.tile([1, K], dt.float32, tag="ccp")
                    for ct in range(CT):
                        sq = sqp.tile([128, K], dt.float32, tag="sq")
                        nc.scalar.activation(sq[:], CENTT[:, ct, :], ACT.Square)
                        nc.tensor.matmul(ccp[0:1, 0:512], ONES1[:], sq[:, 0:512],
                                         start=(ct == 0), stop=(ct == CT - 1))
                        nc.tensor.matmul(ccp[0:1, 512:K], ONES1[:], sq[:, 512:K],
                                         start=(ct == 0), stop=(ct == CT - 1))
                    nc.scalar.activation(CCH[:], ccp[0:1, :], ACT.Copy, scale=0.5)
                with tc.tile_pool(name=f"cb{it}", bufs=1, space="PSUM") as cbp:
                    pcb = cbp.tile([128, 1024], dt.float32, tag="pcb")
                    nc.tensor.matmul(pcb[:, 0:512], ONESR[:], CCH[0:1, 0:512], start=True, stop=True)
                    nc.tensor.matmul(pcb[:, 512:K], ONESR[:], CCH[0:1, 512:K], start=True, stop=True)
                    nc.scalar.copy(CCB[:], pcb[:, 0:K])

                # distances (fp16 3-term) + argmax(S - cc/2) -> labels, one-hot, counts
                with tc.tile_pool(name=f"d{it}", bufs=3, space="PSUM") as dps, \
                     tc.tile_pool(name=f"cn{it}", bufs=1, space="PSUM") as cnp:
                    pcnt = cnp.tile([1, K], dt.float32, tag="pcnt")
                    for t in range(NT):
                        S = dps.tile([128, K], dt.float32, tag="S")
                        tb = slice(t * 128, (t + 1) * 128)
                        pairs = [(XH1, GH1), (XH1, GH2), (XH2, GH1)]
                        for ct in range(CT):
                            for pi, (xh, gh) in enumerate(pairs):
                                st_ = (ct == 0 and pi == 0)
                                sp_ = (ct == CT - 1 and pi == 2)
                                nc.tensor.matmul(S[:, 0:512], xh[:, ct, tb], gh[:, ct, 0:512],
                                                 start=st_, stop=sp_)
                                nc.tensor.matmul(S[:, 512:K], xh[:, ct, tb], gh[:, ct, 512:K],
                                                 start=st_, stop=sp_)
                        dm = medium.tile([128, K], dt.float32, tag="dm")
                        nc.vector.tensor_tensor(dm[:], S[:, 0:K], CCB[:], op.subtract)
                        mx = small.tile([128, 8], dt.float32, tag="mx")
                        mi = small.tile([128, 8], dt.uint32, tag="mi")
                        nc.vector.max(mx[:], dm[:])
                        nc.vector.max_index(mi[:], mx[:], dm[:])
                        nc.vector.tensor_copy(LAB[:, t:t + 1], mi[:, 0:1])
                        nc.vector.tensor_scalar(OH[:, t, :], IOTB[:], LAB[:, t:t + 1], None, op.is_equal)
                        nc.tensor.matmul(pcnt[0:1, 0:512], ONES1H[:], OH[:, t, 0:512],
                                         start=(t == 0), stop=(t == NT - 1))
                        nc.tensor.matmul(pcnt[0:1, 512:K], ONES1H[:], OH[:, t, 512:K],
                                         start=(t == 0), stop=(t == NT - 1))
                    nc.scalar.copy(CNTSB[:], pcnt[0:1, :])

                if it == ITERS:
                    itb.release()
                    break

                # broadcast counts, masks
                CNTB = itb.tile([128, K], dt.float32, tag="CNTB")
                CMB = itb.tile([128, K], dt.float32, tag="CMB")
                MGB = itb.tile([128, K], dt.uint8, tag="MGB")
                with tc.tile_pool(name=f"bc{it}", bufs=1, space="PSUM") as bcp:
                    pb = bcp.tile([128, 1024], dt.float32, tag="pb")
                    nc.tensor.matmul(pb[:, 0:512], ONESR[:], CNTSB[0:1, 0:512], start=True, stop=True)
                    nc.tensor.matmul(pb[:, 512:K], ONESR[:], CNTSB[0:1, 512:K], start=True, stop=True)
                    nc.scalar.copy(CNTB[:], pb[:, 0:K])
                nc.vector.tensor_scalar(CMB[:], CNTB[:], 1.0, None, op.max)
                nc.vector.reciprocal(CMB[:], CMB[:])
                nc.vector.tensor_scalar(MGB[:], CNTB[:], 0.0, None, op.is_gt)

                # centroid sums in transposed layout; update CENTT in place
                with tc.tile_pool(name=f"s{it}", bufs=1, space="PSUM") as spsp:
                    PST = spsp.tile([128, CT, 1024], dt.float32, tag="PST")
                    for t in range(NT):
                        for ct in range(CT):
                            cb = slice(ct * 128, (ct + 1) * 128)
                            for ei, eh in enumerate((EH1, EH2)):
                                st_ = (t == 0 and ei == 0)
                                sp_ = (t == NT - 1 and ei == 1)
                                nc.tensor.matmul(PST[:, ct, 0:512], eh[:, t, cb],
                                                 OH[:, t, 0:512], start=st_, stop=sp_)
                                nc.tensor.matmul(PST[:, ct, 512:K], eh[:, t, cb],
                                                 OH[:, t, 512:K], start=st_, stop=sp_)
                    for ct in range(CT):
                        q = medium.tile([128, K], dt.float32, tag="q")
                        nc.vector.tensor_tensor(q[:], PST[:, ct, 0:K], CMB[:], op.mult)
                        nc.vector.copy_predicated(CENTT[:, ct, :], MGB[:], q[:])
                itb.release()

            # ---- colsum/t2 row accumulations (PE work to overlap the sim sweep)
            ep2 = tc.alloc_tile_pool(name="ep2", bufs=1, space="PSUM")
            cer_p = ep2.tile([1, C], dt.float32, tag="cer_p")
            t2_p = ep2.tile([1, C], dt.float32, tag="t2_p")
            for t in range(NT):
                for ei, eh in enumerate((EH1, EH2)):
                    nc.tensor.matmul(cer_p[0:1, :], ONES1H[:], eh[:, t, :],
                                     start=(t == 0 and ei == 0), stop=(t == NT - 1 and ei == 1))
                t2terms = [(2, EH1), (2, EH2), (3, EH1)]
                for pi, (col, eh) in enumerate(t2terms):
                    nc.tensor.matmul(t2_p[0:1, :], V2H[:, t, col:col + 1], eh[:, t, :],
                                     start=(t == 0 and pi == 0), stop=(t == NT - 1 and pi == 2))

            # ---- build CENT [kk, C] from final CENTT
            with tc.tile_pool(name="tr0", bufs=2, space="PSUM") as trp:
                for r in range(KT):
                    rows = KROWS[r]
                    for ct in range(CT):
                        tp = trp.tile([128, 128], dt.float32, tag="tp")
                        nc.tensor.transpose(tp[0:rows, 0:128], CENTT[:, ct, r * 128:r * 128 + rows],
                                            EYE[:, :])
                        nc.scalar.copy(CENT[0:rows, r, ct * 128:(ct + 1) * 128], tp[0:rows, 0:128])

            # ---- refine
            with tc.tile_pool(name="lrow", bufs=1) as lrp:
                LROW = lrp.tile([1, N], dt.float32, tag="LROW")
                LROWB = lrp.tile([128, N], dt.float32, tag="LROWB")
                for t in range(NT):
                    nc.sync.dma_start(LROW[0:1, t * 128:(t + 1) * 128], LAB[:, t:t + 1])
                with tc.tile_pool(name="pbl", bufs=2, space="PSUM") as pbp:
                    for off, w in [(i * 512, min(512, N - i * 512)) for i in range((N + 511) // 512)]:
                        pb2 = pbp.tile([128, 512], dt.float32, tag="pb2")
                        nc.tensor.matmul(pb2[:, 0:w], ONESR[:], LROW[0:1, off:off + w], start=True, stop=True)
                        nc.scalar.copy(LROWB[:, off:off + w], pb2[:, 0:w])

                # per-point weights w_n = sum_{m in cluster(n)} sim[n, m]
                with tc.tile_pool(name="simst", bufs=2) as sst:
                    for t in range(NT):
                        smt = sst.tile([128, N], dt.float32, tag="simt")
                        nc.sync.dma_start(smt[:], sim_d.ap()[t * 128:(t + 1) * 128, :])
                        nc.vector.scalar_tensor_tensor(smt[:], LROWB[:], LAB[:, t:t + 1], smt[:],
                                                       op.is_equal, op.mult,
                                                       accum_out=WWT[:, t:t + 1])

            with tc.tile_pool(name="ref", bufs=1) as refp:
                # fp16 hi/lo split of per-point weights w into V2H cols 0,1
                AUXSB = refp.tile([4, K], dt.float32, tag="AUXSB")
                nc.vector.tensor_copy(V2H[:, :, 0], WWT[:])
                nc.scalar.copy(WHIF[:], V2H[:, :, 0])
                nc.vector.tensor_tensor(WLOF[:], WWT[:], WHIF[:], op.subtract)
                nc.vector.tensor_copy(V2H[:, :, 1], WLOF[:])

                # pass a: aux rows [whi; wlo; swhi; swlo].T @ oh
                with tc.tile_pool(name="rsa", bufs=1, space="PSUM") as rpsa:
                    AUX = rpsa.tile([4, K], dt.float32, tag="AUX")
                    for t in range(NT):
                        st_, sp_ = (t == 0), (t == NT - 1)
                        nc.tensor.matmul(AUX[0:4, 0:512], V2H[:, t, :], OH[:, t, 0:512],
                                         start=st_, stop=sp_)
                        nc.tensor.matmul(AUX[0:4, 512:K], V2H[:, t, :], OH[:, t, 512:K],
                                         start=st_, stop=sp_)
                    nc.scalar.copy(AUXSB[:], AUX[0:4, :])

                # transpose aux/cnt rows into per-kk columns
                with tc.tile_pool(name="tr1", bufs=2, space="PSUM") as tr1:
                    for r in range(KT):
                        rows = KROWS[r]
                        sl = slice(r * 128, r * 128 + rows)
                        tpa = tr1.tile([128, 4], dt.float32, tag="tpa")
                        nc.tensor.transpose(tpa[0:rows, 0:4], AUXSB[0:4, sl], EYE[0:4, 0:4])
                        nc.scalar.copy(CWS[0:rows, r, 0:4], tpa[0:rows, 0:4])
                        tpc = tr1.tile([128, 1], dt.float32, tag="tpc")
                        nc.tensor.transpose(tpc[0:rows, 0:1], CNTSB[0:1, sl], EYE[0:1, 0:1])
                        nc.scalar.copy(CWS[0:rows, r, 4:5], tpc[0:rows, 0:1])
                nc.vector.tensor_tensor(SSWC[:], CWS[:, :, 2], CWS[:, :, 3], op.add)

                # pass b: weighted sums (kk layout, fp16 splits) + blend
                with tc.tile_pool(name="rsb", bufs=1, space="PSUM") as rpsb:
                    RSUM = rpsb.tile([128, KT, C], dt.float32, tag="RSUM")
                    with tc.tile_pool(name="roh", bufs=3) as rohp:
                        for t in range(NT):
                            ohwh = rohp.tile([128, K], dt.float16, tag="rohwh")
                            ohwl = rohp.tile([128, K], dt.float16, tag="rohwl")
                            nc.vector.tensor_scalar(ohwh[:], IOTB[:], LAB[:, t:t + 1], WHIF[:, t:t + 1],
                                                    op.is_equal, op.mult)
                            nc.vector.tensor_scalar(ohwl[:], IOTB[:], LAB[:, t:t + 1], WLOF[:, t:t + 1],
                                                    op.is_equal, op.mult)
                            st_, sp_ = (t == 0), (t == NT - 1)
                            terms = [(ohwh, EH1), (ohwh, EH2), (ohwl, EH1)]
                            for r in range(KT):
                                rows = KROWS[r]
                                for pi, (ow, eh) in enumerate(terms):
                                    nc.tensor.matmul(RSUM[0:rows, r, :], ow[:, r * 128:r * 128 + rows],
                                                     eh[:, t, :], start=(st_ and pi == 0),
                                                     stop=(sp_ and pi == 2))
                    # centers = where(cnt>0, rsum/(wsum + max(cnt,1)*1e-6), cent)
                    for r in range(KT):
                        rows = KROWS[r]
                        cm = small.tile([128, 1], dt.float32, tag="cm")
                        ws2 = small.tile([128, 1], dt.float32, tag="ws2")
                        wsp = small.tile([128, 1], dt.float32, tag="wsp")
                        mg = small.tile([128, 1], dt.float32, tag="mg")
                        dd = medium.tile([128, C], dt.float32, tag="dd")
                        nc.vector.tensor_scalar(cm[0:rows], CWS[0:rows, r, 4:5], 1.0, None, op.max)
                        nc.vector.tensor_tensor(ws2[0:rows], CWS[0:rows, r, 0:1], CWS[0:rows, r, 1:2], op.add)
                        nc.vector.scalar_tensor_tensor(wsp[0:rows], cm[0:rows], 1e-6,
                                                       ws2[0:rows, 0:1], op.mult, op.add)
                        nc.vector.reciprocal(wsp[0:rows], wsp[0:rows])
                        nc.vector.tensor_scalar(mg[0:rows], CWS[0:rows, r, 4:5], 0.0, None, op.is_gt)
                        nc.vector.scalar_tensor_tensor(dd[0:rows, :], RSUM[0:rows, r, :], wsp[0:rows, 0:1],
                                                       CENT[0:rows, r, :], op.mult, op.subtract)
                        nc.vector.scalar_tensor_tensor(CENT[0:rows, r, :], dd[0:rows, :], mg[0:rows, 0:1],
                                                       CENT[0:rows, r, :], op.mult, op.add)
                        nc.sync.dma_start(cent_d.ap()[r * 128:r * 128 + rows, :], CENT[0:rows, r, :])

            # ---- epilogue: means + fusion gate (row layout)
            with tc.tile_pool(name="epb", bufs=1) as epb, \
                 tc.tile_pool(name="ep", bufs=1, space="PSUM") as eps:
                FUW1 = epb.tile([128, 8, C2], dt.float32, tag="FUW1")
                FUB1 = epb.tile([128, 2], dt.float32, tag="FUB1")
                FUW2 = epb.tile([128, 2], dt.float32, tag="FUW2")
                FUB2 = epb.tile([1, 1], dt.float32, tag="FUB2")
                nc.sync.dma_start(FUW1[:], fuw1_d.ap())
                nc.sync.dma_start(FUB1[:], fub1_d.ap())
                nc.sync.dma_start(FUW2[:], fuw2_d.ap())
                nc.sync.dma_start(FUB2[:], fub2_d.ap())
                CER = epb.tile([1, C], dt.float32, tag="CER")
                GROW = epb.tile([1, C], dt.float32, tag="GROW")
                LROW_ = epb.tile([1, C], dt.float32, tag="LROW_")
                T2R = epb.tile([1, C], dt.float32, tag="T2R")
                T3R = epb.tile([1, C], dt.float32, tag="T3R")
                OROW = epb.tile([1, C], dt.float32, tag="OROW")
                GBUF = epb.tile([128, 8], dt.float32, tag="GBUF")
                H2 = epb.tile([128, 2], dt.float32, tag="H2")
                ASB = epb.tile([1, 1], dt.float32, tag="ASB")
                dgl = epb.tile([1, C], dt.float32, tag="dgl")
                t3_p = eps.tile([1, C], dt.float32, tag="t3_p")
                for r in range(KT):
                    rows = KROWS[r]
                    nc.tensor.matmul(t3_p[0:1, :], SSWC[0:rows, r:r + 1], CENT[0:rows, r, :],
                                     start=(r == 0), stop=(r == KT - 1))
                nc.scalar.copy(CER[:], cer_p[0:1, :])
                nc.scalar.copy(T2R[:], t2_p[0:1, :])
                nc.scalar.copy(T3R[:], t3_p[0:1, :])
                # global = (cer - psr)/N ; local = (cer + t3 - t2)/N
                nc.vector.tensor_tensor(GROW[:], CER[:], PSR[:], op.subtract)
                nc.vector.tensor_scalar(GROW[:], GROW[:], 1.0 / N, None, op.mult)
                nc.vector.tensor_tensor(LROW_[:], CER[:], T3R[:], op.add)
                nc.vector.tensor_tensor(LROW_[:], LROW_[:], T2R[:], op.subtract)
                nc.vector.tensor_scalar(LROW_[:], LROW_[:], 1.0 / N, None, op.mult)

                # transpose [g | l] rows into GBUF columns (K-layout for fusion)
                with tc.tile_pool(name="tr2", bufs=2, space="PSUM") as tr2:
                    for j in range(8):
                        src = GROW if j < 4 else LROW_
                        o = (j % 4) * 128
                        tpg = tr2.tile([128, 1], dt.float32, tag="tpg")
                        nc.tensor.transpose(tpg[0:128, 0:1], src[0:1, o:o + 128], EYE[0:1, 0:1])
                        nc.scalar.copy(GBUF[:, j:j + 1], tpg[:, 0:1])

                pfu = eps.tile([128, 2, 512], dt.float32, tag="pfu")
                pa = eps.tile([1, 1], dt.float32, tag="pa")
                for m in range(2):
                    for kt in range(8):
                        nc.tensor.matmul(pfu[:, m, 0:1], FUW1[:, kt, m * 128:(m + 1) * 128],
                                         GBUF[:, kt:kt + 1], start=(kt == 0), stop=(kt == 7))
                    nc.scalar.activation(H2[:, m:m + 1], pfu[:, m, 0:1], ACT.Relu, bias=FUB1[:, m:m + 1])
                for m in range(2):
                    nc.tensor.matmul(pa[0:1, 0:1], H2[:, m:m + 1], FUW2[:, m:m + 1],
                                     start=(m == 0), stop=(m == 1))
                nc.scalar.activation(ASB[:], pa[0:1, 0:1], ACT.Sigmoid, bias=FUB2[0:1, 0:1])
                # out = l + a*(g - l)
                nc.vector.tensor_tensor(dgl[:], GROW[:], LROW_[:], op.subtract)
                nc.vector.scalar_tensor_tensor(OROW[:], dgl[:], ASB[0:1, 0:1], LROW_[:], op.mult, op.add)
                nc.sync.dma_start(outv_d.ap(), OROW[:])
            ep2.release()

        pers.release()

    nc.compile()
    return nc


def _get_program():
    if "nc" not in _PROG:
        _PROG["nc"] = _build_program()
    return _PROG["nc"]


# ------------------------------------------------------------------- kernel
def _prep_in_maps(features, params):
    feats = np.asarray(features, F32)
    p = {k: np.asarray(v, F32) for k, v in params.items()}

    pos = _host_pos_emb(p)                       # [N, C] f32
    post = np.ascontiguousarray(pos.T)           # [C, N]
    sim = _host_sim()                            # [N, N]
    idx = _host_idx()                            # [K]
    iotab = np.broadcast_to(np.arange(K, dtype=F32), (128, K)).copy()
    eye = np.eye(128, dtype=F32)

    spw1 = np.ascontiguousarray(p["sp_w1"].reshape(4, 128, C2).transpose(1, 0, 2))
    spb1 = np.ascontiguousarray(p["sp_b1"].reshape(2, 128).T)
    spw2 = np.ascontiguousarray(p["sp_w2"].reshape(2, 128).T)
    spb2 = np.full((128, 1), p["sp_b2"][0], F32)
    fuw1 = np.ascontiguousarray(p["fu_w1"].reshape(8, 128, C2).transpose(1, 0, 2))
    fub1 = np.ascontiguousarray(p["fu_b1"].reshape(2, 128).T)
    fuw2 = np.ascontiguousarray(p["fu_w2"].reshape(2, 128).T)
    fub2 = np.full((1, 1), p["fu_b2"][0], F32)

    shared = dict(post=post, pos=pos, sim=sim, iotab=iotab, eye=eye,
                  spw1=spw1, spb1=spb1, spw2=spw2, spb2=spb2,
                  fuw1=fuw1, fub1=fub1, fuw2=fuw2, fub2=fub2)

    in_maps = []
    for b in range(B):
        xt = np.ascontiguousarray(feats[b].reshape(C, N))
        x = np.ascontiguousarray(xt.T)
        xe = x + pos
        c0t = np.ascontiguousarray(xe[idx].T)
        in_maps.append(dict(xt=xt, x=x, c0t=c0t, **shared))
    return in_maps


def kernel(features, params):
    from concourse import bass_utils

    nc = _get_program()
    in_maps = _prep_in_maps(features, params)
    res = bass_utils.run_bass_kernel_spmd(nc, in_maps, core_ids=list(range(B)))
    if res.exec_time_ns is not None:
        print(f"HW exec time: {res.exec_time_ns} ns")

    out = np.empty((B, C), F32)
    centers = np.empty((B, K, C), F32)
    for b, r in enumerate(res.results):
        out[b] = r["outv"].reshape(C)
        centers[b] = r["centers"]
    return out, centers
